# revision 23
# baseline (speedup 1.0000x reference)
"""BinarizeConv2dSDP kernel for Trainium2 (8 NeuronCores, data-parallel over batch).

out = conv2d(sign(x), sign(M + sum_k rv[k] * Z[k]), stride 1, pad 1) * Alpha

The reference's rsqrt pre-normalization is strictly positive and multiplicative,
so sign(w) is unaffected: binary weights are sign(M + rv@Z).

v2 strategy (from v1's 90us trace analysis):
  - fp16 I/O on the device: x is uploaded as fp16 (sign(fp16(x)) == sign(x) for
    all practically occurring values) and out is stored as fp16 (conv counts
    <= 1152 are fp16-exact; only the Alpha multiply rounds, ~5e-4 rel err).
    This halves the dominant HBM traffic: 29.2MB -> 16.4MB per core against
    the ~435 GB/s aggregate DMA cap.
  - p-outer conv loop: the 5 weight pairs each sweep all 7 row-chunks of an
    image into 7 PSUM banks, so consecutive matmuls share the stationary
    weights; redundant per-matmul LDWEIGHTS (41.6us of v1's tensor time) are
    elided via InstMatmult.ldweights=False on the 6 followers of each pass.
  - PSUM->SBUF evictions (with per-channel Alpha scale) split across DVE
    (chunks 0,2,4,6) and ScalarE (1,3,5); GpSimd cannot read PSUM.
  - weight-gen STT chain split in half across DVE and GpSimd.
  - pad-border memsets done once per physical ba buffer (3 bufs, manual
    rotation) instead of per image.
  - input x on the SP HWDGE ring, Z/M behind x0 (weight gen gates the first
    conv), alpha/rv on the ACT ring; outputs ride SWDGE except the final
    image's two tail pieces which use the then-idle SP/ACT rings.
"""

import numpy as np
from contextlib import ExitStack

import concourse.bass as bass
import concourse.bass_utils as _bass_utils
import concourse.mybir as mybir
import concourse.tile as tile
from concourse.bacc import Bacc
from concourse.bass_utils import run_bass_kernel_spmd

# Rewrite --enable-ldw-opt on the walrus_driver invocation.  Disabled: the
# walrus pass rejects the standalone InstLdweights that tile_legalize emits
# ("InstLdweights is not compatible with LDW optimization"); the redundant
# loads are excised directly in _excise_redundant_ldweights instead.
LDW_OPT = False


def _patch_walrus_ldw_opt():
    if getattr(_bass_utils, "_ldw_opt_patched", False):
        return
    orig = _bass_utils.run_command

    def run_command_ldw(argv, **kwargs):
        if (
            LDW_OPT
            and isinstance(argv, list)
            and any("walrus_driver" in str(a) for a in argv)
        ):
            argv = [
                "--enable-ldw-opt=true" if a == "--enable-ldw-opt=false" else a
                for a in argv
            ]
        return orig(argv, **kwargs)

    _bass_utils.run_command = run_command_ldw
    _bass_utils._ldw_opt_patched = True


_patch_walrus_ldw_opt()

N_CORES = 8
B, C, H, W = 64, 128, 56, 56
BPC = B // N_CORES  # images per core
KS, K = 3, 5
PH, PW = H + 2, W + 2  # zero-padded image
CHUNK_ROWS = 8
N_CHUNKS = H // CHUNK_ROWS
FREE = CHUNK_ROWS * W  # valid output elements per chunk (448)
FREE_R = CHUNK_ROWS * PW  # matmul free dim incl. garbage cols (464 <= 512)
F32 = mybir.dt.float32
F16 = mybir.dt.float16
BF16 = mybir.dt.bfloat16
F8 = mybir.dt.float8e4

# Elide the redundant LDWEIGHTS on matmuls 2..7 of each weight pass (the
# stationary tile is unchanged within a pass).
LDW_ELIDE = True


def _excise_redundant_ldweights(nc):
    """Remove InstLdweights whose stationary tile is already resident.

    tile_legalize pairs every non-f32 InstMatmult with a standalone
    InstLdweights; in the p-outer conv loop 6 of every 7 reload the identical
    weights.  Walrus emits no weight load for an InstMatmult with
    ldweights=False when its standalone load is gone, so the PE array keeps
    the resident weights and the matmul cadence drops from ~236ns
    (LDWEIGHTS-bound) to the raw fp8 DoubleRow rate.  Waits/updates of a
    removed load are merged into the following matmul; Bacc.compile()
    legalizes any wait overflow afterwards.
    """
    import concourse.mybir as _mb

    removed = 0
    for blk in nc.main_func.blocks:
        insts = list(blk.instructions)
        last_sig = None
        keep = []
        for idx, inst in enumerate(insts):
            if isinstance(inst, _mb.InstLdweights):
                a = inst.ins[0]
                sig = (
                    a.memref,
                    a.offset,
                    tuple(tuple(p) for p in a.ap),
                    str(a.dtype),
                )
                nxt = insts[idx + 1] if idx + 1 < len(insts) else None
                if (
                    sig == last_sig
                    and isinstance(nxt, _mb.InstMatmult)
                    and not nxt.is_transpose
                ):
                    si = inst.sync_info
                    if si is not None and (si.on_wait or si.on_update):
                        msi = nxt.sync_info
                        if msi is None:
                            nxt.sync_info = _mb.SyncInfo(
                                on_wait=list(si.on_wait),
                                on_update=list(si.on_update),
                            )
                        else:
                            nxt.sync_info = _mb.SyncInfo(
                                on_wait=list(msi.on_wait) + list(si.on_wait),
                                on_update=list(msi.on_update) + list(si.on_update),
                            )
                    try:
                        nxt.merge_dependencies_from(inst)
                    except Exception:
                        pass
                    removed += 1
                    continue  # drop this reload
                last_sig = sig
            elif isinstance(inst, _mb.InstMatmult):
                if inst.is_transpose:
                    last_sig = None
            keep.append(inst)
        if removed and len(keep) != len(insts):
            del blk.instructions[:]
            for inst in keep:
                blk.instructions.append(inst)
    return removed


def build_kernel(rv_vals):
    """Build the single-core Bass module (SPMD: same program on all 8 cores).

    rv_vals: the 5 rv scalars, baked as immediates into the weight-gen ops.
    """
    nc = Bacc()
    OCS = C // N_CORES  # out-channel slice per core for weight generation
    x_p = nc.declare_dram_parameter("x", [BPC, C, H, W], F16, isOutput=False)
    m_p = nc.declare_dram_parameter("M", [OCS, C, KS, KS], F32, isOutput=False)
    z_p = nc.declare_dram_parameter("Z", [K, OCS, C, KS, KS], F32, isOutput=False)
    a_p = nc.declare_dram_parameter("Alpha", [C, 1, 1], F32, isOutput=False)
    rv_p = nc.declare_dram_parameter("rv", [1, K], F32, isOutput=False)
    out_p = nc.declare_dram_parameter("out", [BPC, C, H, W], F16, isOutput=True)

    NW = C * KS * KS  # 1152 weight elements per out-channel row
    HALF = (H // 2) * W  # first-half image elements (28 rows)
    XF = C // OCS  # partition-packing factor for the weight-gen slice (8)
    NWS = NW // XF  # free elems per packed partition row (144)

    with tile.TileContext(nc) as tc, ExitStack() as ctx:
        const = ctx.enter_context(tc.tile_pool(name="const", bufs=1))
        wg = ctx.enter_context(tc.tile_pool(name="wg", bufs=1))
        zpool = ctx.enter_context(tc.tile_pool(name="zpool", bufs=1))
        xin = ctx.enter_context(tc.tile_pool(name="xin", bufs=BPC))
        pad = ctx.enter_context(tc.tile_pool(name="pad", bufs=1))
        opool = ctx.enter_context(tc.tile_pool(name="opool", bufs=3))
        ps = ctx.enter_context(tc.tile_pool(name="ps", bufs=1, space="PSUM"))
        dram = ctx.enter_context(tc.tile_pool(name="dram", bufs=1, space="DRAM"))

        # ---- constants ----
        # Anti-diagonal permutation: transpose against it yields the transposed
        # tap with REVERSED out-channel columns, which is exactly the column
        # order DoubleRowSwInterleave's weight layout wants.
        # The tile name doubles as a NEFF-cache marker for the ldw-opt flag
        # (the cache keys on BIR content, not compiler flags).
        identity = const.tile([C, C], BF16, name=f"identity_ldw{int(LDW_OPT)}")
        nc.gpsimd.memset(identity[:], 0.0)
        nc.gpsimd.affine_select(
            out=identity[:],
            in_=identity[:],
            compare_op=mybir.AluOpType.not_equal,
            fill=1.0,
            base=-(C - 1),
            pattern=[[1, C]],
            channel_multiplier=1,
        )
        # Alpha/rv ride the ACT HWDGE ring so the SP ring's first slots
        # belong to x0/Z.
        alpha_sb = const.tile([C, 1], F32)
        nc.scalar.dma_start(alpha_sb[:], a_p[:].rearrange("c a b -> c (a b)"))
        rv_sb = const.tile([1, K], F32)
        nc.scalar.dma_start(rv_sb[:], rv_p[:])

        x_ap = x_p[:]
        o_ap = out_p[:]

        # The HWDGE SP ring drains FIFO, so this issue order is the wire
        # order.  The weight-gen STT chain paces to z_k arrivals (one STT per
        # landing), so Z goes out early, interleaved with image 0's halves
        # (whose signs must finish before the conv, ~16us in).  All remaining
        # images are issued up front (xin has BPC bufs, nothing recycles, the
        # ring never starves).
        # Each core only loads its 16-out-channel slice of Z/M (the host
        # shards them rank-major); the generated binary-weight slices are
        # all-gathered below.  The slice is packed 8 ic-groups per partition
        # so all 128 partitions work: [OCS, NW] -> [(o x), NW/x].
        x_sbs = []
        x_sbs.append(xin.tile([C, H * W], F16, name="x_sb0", tag="x_sb"))
        nc.sync.dma_start(
            x_sbs[0][:, 0:HALF], x_ap[0].rearrange("c h w -> c (h w)")[:, 0:HALF]
        )
        m_sb = wg.tile([C, NWS], F32)
        nc.sync.dma_start(
            m_sb[:],
            m_p[:].rearrange("o i a b -> o (i a b)").rearrange(
                "o (x n) -> (o x) n", x=XF
            ),
        )
        z_sbs = []
        for k in range(K):
            z_sbs.append(zpool.tile([C, NWS], F32, name=f"z{k}", tag=f"z{k}"))
            nc.sync.dma_start(
                z_sbs[k][:],
                z_p[k].rearrange("o i a b -> o (i a b)").rearrange(
                    "o (x n) -> (o x) n", x=XF
                ),
            )
            if k == 1:
                nc.sync.dma_start(
                    x_sbs[0][:, HALF:],
                    x_ap[0].rearrange("c h w -> c (h w)")[:, HALF:],
                )
        for i in range(1, BPC):
            x_sbs.append(xin.tile([C, H * W], F16, name=f"x_sb{i}", tag="x_sb"))
            nc.sync.dma_start(
                x_sbs[i][:], x_ap[i].rearrange("c h w -> c (h w)")
            )

        # ---- padded sign buffers: 3 physical buffers, borders zeroed ONCE.
        # Every image only writes the interior, so the zero border persists
        # across reuses.
        ba_bufs = []
        for b in range(3):
            ba = pad.tile([C, PH * PW + 2], F8, name=f"ba{b}", tag=f"ba{b}")
            ba_r = ba[:, 0 : PH * PW].rearrange("c (h w) -> c h w", w=PW)
            nc.gpsimd.memset(ba[:, 0:PW], 0.0)
            nc.gpsimd.memset(ba[:, (PH - 1) * PW : PH * PW + 2], 0.0)
            nc.gpsimd.memset(ba_r[:, 1 : H + 1, 0:1], 0.0)
            nc.gpsimd.memset(ba_r[:, 1 : H + 1, W + 1 : PW], 0.0)
            ba_bufs.append(ba)

        # ---- weight generation: w = M + sum_k rv_k Z_k on this core's
        # out-channel slice (DVE; Pool lacks the TensorScalarPtr op).  M
        # folds into the first STT so the chain is 5 tiny [128,144] ops.
        w_sb = wg.tile([C, NWS], F32)
        nc.vector.scalar_tensor_tensor(
            w_sb[:],
            z_sbs[0][:],
            float(rv_vals[0]),
            m_sb[:],
            mybir.AluOpType.mult,
            mybir.AluOpType.add,
        )
        for k in range(1, K):
            nc.vector.scalar_tensor_tensor(
                w_sb[:],
                z_sbs[k][:],
                float(rv_vals[k]),
                w_sb[:],
                mybir.AluOpType.mult,
                mybir.AluOpType.add,
            )
        bw_sl = wg.tile([C, NWS], BF16)
        bw_sb = wg.tile([C, NW], BF16)

        def psum_tile(ch, shape, dtype, name):
            # pt0 gets 2 banks so image i+1's first matmul needn't wait for
            # image i's chunk-0 eviction; 2 + 6 = 8 banks total.
            return ps.tile(
                shape, dtype, name=name, tag=f"pt{ch}", bufs=(2 if ch == 0 else 1)
            )

        def sign_image(i, halves=False):
            """Binarize image i's fp16 pixels into its ba buffer interior."""
            ba = ba_bufs[i % 3]
            ba_r = ba[:, 0 : PH * PW].rearrange("c (h w) -> c h w", w=PW)
            x_r = x_sbs[i][:].rearrange("c (h w) -> c h w", w=W)
            if halves:
                nc.scalar.sign(ba_r[:, 1 : H // 2 + 1, 1 : W + 1], x_r[:, : H // 2])
                nc.scalar.sign(ba_r[:, H // 2 + 1 : H + 1, 1 : W + 1], x_r[:, H // 2 :])
            else:
                nc.scalar.sign(ba_r[:, 1 : H + 1, 1 : W + 1], x_r)
            return ba

        # Image 0's sign runs as soon as its half-DMAs land, before the
        # weight sign (which waits on the Z chain) enters the ACT queue.
        sign_image(0, halves=True)
        nc.scalar.sign(bw_sl[:], w_sb[:])
        sign_image(1)

        # All-gather the 8 cores' binary-weight slices (bf16, 36.9KB per
        # rank) through HBM bounce buffers; every core then holds the full
        # [C, NW] binary weights.
        bw_in = dram.tile([OCS, NW], BF16, name="bw_in")
        bw_all = dram.tile([C, NW], BF16, name="bw_all", addr_space="Shared")
        nc.gpsimd.dma_start(
            bw_in[:].rearrange("o (x n) -> (o x) n", x=XF), bw_sl[:]
        )
        nc.gpsimd.collective_compute(
            "AllGather",
            mybir.AluOpType.bypass,
            replica_groups=[list(range(N_CORES))],
            ins=[bw_in[:].opt()],
            outs=[bw_all[:].opt()],
        )
        nc.gpsimd.dma_start(bw_sb[:], bw_all[:])

        # Transpose each tap's [oc, ic] into [ic, oc-reversed] (via the
        # anti-diagonal permutation), then interleave tap pairs column-wise as
        # fp8e4 (+-1 exact): the DoubleRowSwInterleave weight layout.  The
        # transposes borrow the conv's PSUM banks (idle during the prologue).
        # Chain every PE matmul (transposes included) in emission order with
        # ordering-only deps: the tile scheduler otherwise interleaves the
        # weight passes, breaking the same-weights runs the LDWEIGHTS
        # excision needs.
        pe_chain = [None]

        from concourse.instruction_name_ordered_set import (
            InstructionNameOrderedSet,
        )

        def chain_pe(bi):
            raw = bi.ins
            if pe_chain[0] is not None:
                s = InstructionNameOrderedSet()
                s.add(pe_chain[0])
                raw.add_nosync_dependencies_from(s)
            pe_chain[0] = raw.name

        wt = const.tile([C, 5, 2 * C], F8)
        nc.vector.memset(wt[:, 4, :], 0.0)
        bw_r = bw_sb[:].rearrange("o (i j) -> o i j", j=KS * KS)
        for j in range(KS * KS):
            tp = psum_tile(j % N_CHUNKS, [C, C], BF16, f"tp{j}")
            chain_pe(nc.tensor.transpose(tp[:], bw_r[:, :, j], identity[:]))
            pair, slot = divmod(j, 2)
            wt_h = wt[:].tensor
            dst = bass.AP(wt_h, pair * 2 * C + slot, [[5 * 2 * C, C], [2, C]])
            nc.vector.tensor_copy(dst, tp[:])
        # rv reaches the kernel as baked immediates; touch the tensor so the
        # bound input isn't dead.
        nc.vector.tensor_copy(w_sb[0:1, 0:K], rv_sb[0:1, :])

        def tap_off(r0, j):
            # flat offset of (out-row r0, tap j)'s top-left read in the padded image
            if j == KS * KS:  # zero tap: alias tap 8's window (weights are 0)
                j = KS * KS - 1
            return (r0 + j // KS) * PW + (j % KS)

        # Eviction engine per chunk: GpSimd has no PSUM port, so split
        # DVE/ScalarE; ScalarE also carries the signs.
        EVICT_DVE = (0, 2, 4, 6)

        def conv_image(i, ba):
            """5 weight passes x 7 chunk matmuls into 7 PSUM banks, then
            alpha-scaled eviction to fp16 and the output DMA."""
            pts = [
                psum_tile(ch, [C, 512], F32, f"pt{ch}_{i}") for ch in range(N_CHUNKS)
            ]
            for p in range(5):
                for ch in range(N_CHUNKS):
                    r0 = ch * CHUNK_ROWS
                    o0 = tap_off(r0, 2 * p)
                    o1 = tap_off(r0, 2 * p + 1)
                    rhs = bass.AP(
                        ba[:].tensor,
                        o0,
                        [[PH * PW + 2, C], [o1 - o0, 2], [1, FREE_R]],
                    )
                    mi = nc.tensor.matmul(
                        pts[ch][:, 0:FREE_R],
                        wt[:, p, :],
                        rhs,
                        start=(p == 0),
                        stop=(p == 4),
                        perf_mode=mybir.MatmulPerfMode.DoubleRowSwInterleave,
                    )
                    chain_pe(mi)
            o_sb = opool.tile([C, H * W], F16, name=f"o_sb{i}", tag="o_sb")
            for ch in range(N_CHUNKS):
                eng = nc.vector if ch in EVICT_DVE else nc.scalar
                src = pts[ch][:, 0:FREE_R].rearrange("c (a b) -> c a b", b=PW)[
                    :, :, 0:W
                ]
                dst = o_sb[:, ch * FREE : (ch + 1) * FREE].rearrange(
                    "c (a b) -> c a b", b=W
                )
                if ch in EVICT_DVE:
                    eng.tensor_scalar_mul(dst, src, alpha_sb[:, 0:1])
                else:
                    eng.mul(dst, src, alpha_sb[:, 0:1])
            return o_sb

        # Software-pipelined image loop.  Signs for images 0/1 were emitted
        # above; each iteration's sign(i+2) is emitted AFTER image i's
        # ScalarE evictions so it doesn't delay them in the ACT queue.
        for i in range(BPC):
            o_sb = conv_image(i, ba_bufs[i % 3])
            if i + 2 < BPC:
                sign_image(i + 2)
            o_hbm = o_ap[i].rearrange("c h w -> c (h w)")
            if i < 5:
                # Early outputs ride SWDGE (GpSimd) so they never head-of-line
                # block input loads on the FIFO HWDGE SP ring.
                nc.gpsimd.dma_start(o_hbm, o_sb[:])
            elif i < BPC - 1:
                # All input issues are done by ~20us; the SP ring is idle.
                nc.sync.dma_start(o_hbm, o_sb[:])
            else:
                # Final image drains in 2-chunk pieces, alternating the two
                # idle HWDGE rings, each issued as soon as its chunks evict.
                nc.sync.dma_start(o_hbm[:, 0 : 2 * FREE], o_sb[:, 0 : 2 * FREE])
                nc.scalar.dma_start(
                    o_hbm[:, 2 * FREE : 4 * FREE], o_sb[:, 2 * FREE : 4 * FREE]
                )
                nc.sync.dma_start(
                    o_hbm[:, 4 * FREE : 6 * FREE], o_sb[:, 4 * FREE : 6 * FREE]
                )
                nc.scalar.dma_start(o_hbm[:, 6 * FREE :], o_sb[:, 6 * FREE :])

    if LDW_ELIDE:
        _excise_redundant_ldweights(nc)
    nc.finalize()
    return nc


_CACHE = {}


def _get_nc(rv):
    key = rv.tobytes()
    if key not in _CACHE:
        _CACHE[key] = build_kernel(np.asarray(rv, np.float32).reshape(-1))
    return _CACHE[key]


def _run(inputs, trace=False):
    x = np.ascontiguousarray(np.asarray(inputs["x"]), dtype=np.float16)
    M = np.ascontiguousarray(np.asarray(inputs["M"], np.float32))
    Z = np.ascontiguousarray(np.asarray(inputs["Z"], np.float32))
    Alpha = np.ascontiguousarray(np.asarray(inputs["Alpha"], np.float32))
    rv = np.ascontiguousarray(np.asarray(inputs["rv"], np.float32))
    nc = _get_nc(rv)
    OCS = C // N_CORES
    in_maps = [
        {
            "x": x[c * BPC : (c + 1) * BPC],
            "M": np.ascontiguousarray(M[c * OCS : (c + 1) * OCS]),
            "Z": np.ascontiguousarray(Z[:, c * OCS : (c + 1) * OCS]),
            "Alpha": Alpha,
            "rv": rv,
        }
        for c in range(N_CORES)
    ]
    res = run_bass_kernel_spmd(nc, in_maps, list(range(N_CORES)), trace=trace)
    out = np.concatenate([res.results[c]["out"] for c in range(N_CORES)], axis=0)
    return np.asarray(out, dtype=np.float32), res


def kernel(**inputs):
    out, _ = _run(inputs, trace=False)
    return out


def kernel_traced(**inputs):
    out, res = _run(inputs, trace=True)
    return out, res


# revision 24
# speedup vs baseline: 1.8568x; 1.8568x over previous
"""BinarizeConv2dSDP kernel for Trainium2 (8 NeuronCores, data-parallel over batch).

out = conv2d(sign(x), sign(M + sum_k rv[k] * Z[k]), stride 1, pad 1) * Alpha

The reference's rsqrt pre-normalization is strictly positive and multiplicative,
so sign(w) is unaffected: binary weights are sign(M + rv@Z).

v2 strategy (from v1's 90us trace analysis):
  - fp16 I/O on the device: x is uploaded as fp16 (sign(fp16(x)) == sign(x) for
    all practically occurring values) and out is stored as fp16 (conv counts
    <= 1152 are fp16-exact; only the Alpha multiply rounds, ~5e-4 rel err).
    This halves the dominant HBM traffic: 29.2MB -> 16.4MB per core against
    the ~435 GB/s aggregate DMA cap.
  - p-outer conv loop: the 5 weight pairs each sweep all 7 row-chunks of an
    image into 7 PSUM banks, so consecutive matmuls share the stationary
    weights; redundant per-matmul LDWEIGHTS (41.6us of v1's tensor time) are
    elided via InstMatmult.ldweights=False on the 6 followers of each pass.
  - PSUM->SBUF evictions (with per-channel Alpha scale) split across DVE
    (chunks 0,2,4,6) and ScalarE (1,3,5); GpSimd cannot read PSUM.
  - weight-gen STT chain split in half across DVE and GpSimd.
  - pad-border memsets done once per physical ba buffer (3 bufs, manual
    rotation) instead of per image.
  - input x on the SP HWDGE ring, Z/M behind x0 (weight gen gates the first
    conv), alpha/rv on the ACT ring; outputs ride SWDGE except the final
    image's two tail pieces which use the then-idle SP/ACT rings.
"""

import numpy as np
from contextlib import ExitStack

import concourse.bass as bass
import concourse.bass_utils as _bass_utils
import concourse.mybir as mybir
import concourse.tile as tile
from concourse.bacc import Bacc
from concourse.bass_utils import run_bass_kernel_spmd

# Rewrite --enable-ldw-opt on the walrus_driver invocation.  Disabled: the
# walrus pass rejects the standalone InstLdweights that tile_legalize emits
# ("InstLdweights is not compatible with LDW optimization"); the redundant
# loads are excised directly in _excise_redundant_ldweights instead.
LDW_OPT = False


def _patch_walrus_ldw_opt():
    if getattr(_bass_utils, "_ldw_opt_patched", False):
        return
    orig = _bass_utils.run_command

    def run_command_ldw(argv, **kwargs):
        if (
            LDW_OPT
            and isinstance(argv, list)
            and any("walrus_driver" in str(a) for a in argv)
        ):
            argv = [
                "--enable-ldw-opt=true" if a == "--enable-ldw-opt=false" else a
                for a in argv
            ]
        return orig(argv, **kwargs)

    _bass_utils.run_command = run_command_ldw
    _bass_utils._ldw_opt_patched = True


_patch_walrus_ldw_opt()

N_CORES = 8
B, C, H, W = 64, 128, 56, 56
BPC = B // N_CORES  # images per core
KS, K = 3, 5
PH, PW = H + 2, W + 2  # zero-padded image
CHUNK_ROWS = 8
N_CHUNKS = H // CHUNK_ROWS
FREE = CHUNK_ROWS * W  # valid output elements per chunk (448)
FREE_R = CHUNK_ROWS * PW  # matmul free dim incl. garbage cols (464 <= 512)
F32 = mybir.dt.float32
F16 = mybir.dt.float16
BF16 = mybir.dt.bfloat16
F8 = mybir.dt.float8e4

# Elide the redundant LDWEIGHTS on matmuls 2..7 of each weight pass (the
# stationary tile is unchanged within a pass).
LDW_ELIDE = True


def _excise_redundant_ldweights(nc):
    """Remove InstLdweights whose stationary tile is already resident.

    tile_legalize pairs every non-f32 InstMatmult with a standalone
    InstLdweights; in the p-outer conv loop 6 of every 7 reload the identical
    weights.  Walrus emits no weight load for an InstMatmult with
    ldweights=False when its standalone load is gone, so the PE array keeps
    the resident weights and the matmul cadence drops from ~236ns
    (LDWEIGHTS-bound) to the raw fp8 DoubleRow rate.  Waits/updates of a
    removed load are merged into the following matmul; Bacc.compile()
    legalizes any wait overflow afterwards.
    """
    import concourse.mybir as _mb

    removed = 0
    for blk in nc.main_func.blocks:
        insts = list(blk.instructions)
        last_sig = None
        keep = []
        for idx, inst in enumerate(insts):
            if isinstance(inst, _mb.InstLdweights):
                a = inst.ins[0]
                sig = (
                    a.memref,
                    a.offset,
                    tuple(tuple(p) for p in a.ap),
                    str(a.dtype),
                )
                nxt = insts[idx + 1] if idx + 1 < len(insts) else None
                if (
                    sig == last_sig
                    and isinstance(nxt, _mb.InstMatmult)
                    and not nxt.is_transpose
                ):
                    si = inst.sync_info
                    if si is not None and (si.on_wait or si.on_update):
                        msi = nxt.sync_info
                        if msi is None:
                            nxt.sync_info = _mb.SyncInfo(
                                on_wait=list(si.on_wait),
                                on_update=list(si.on_update),
                            )
                        else:
                            nxt.sync_info = _mb.SyncInfo(
                                on_wait=list(msi.on_wait) + list(si.on_wait),
                                on_update=list(msi.on_update) + list(si.on_update),
                            )
                    try:
                        nxt.merge_dependencies_from(inst)
                    except Exception:
                        pass
                    removed += 1
                    continue  # drop this reload
                last_sig = sig
            elif isinstance(inst, _mb.InstMatmult):
                if inst.is_transpose:
                    last_sig = None
            keep.append(inst)
        if removed and len(keep) != len(insts):
            del blk.instructions[:]
            for inst in keep:
                blk.instructions.append(inst)
    return removed


def build_kernel(rv_vals):
    """Build the single-core Bass module (SPMD: same program on all 8 cores).

    rv_vals: the 5 rv scalars, baked as immediates into the weight-gen ops.
    """
    nc = Bacc()
    x_p = nc.declare_dram_parameter("x", [BPC, C, H, W], F16, isOutput=False)
    m_p = nc.declare_dram_parameter("M", [C, C, KS, KS], F32, isOutput=False)
    z_p = nc.declare_dram_parameter("Z", [K, C, C, KS, KS], F32, isOutput=False)
    a_p = nc.declare_dram_parameter("Alpha", [C, 1, 1], F32, isOutput=False)
    rv_p = nc.declare_dram_parameter("rv", [1, K], F32, isOutput=False)
    out_p = nc.declare_dram_parameter("out", [BPC, C, H, W], F16, isOutput=True)

    NW = C * KS * KS  # 1152 weight elements per out-channel row
    HALF = (H // 2) * W  # first-half image elements (28 rows)
    HGEN = NW // 2  # weight-gen column split point (DVE | GpSimd)

    with tile.TileContext(nc) as tc, ExitStack() as ctx:
        const = ctx.enter_context(tc.tile_pool(name="const", bufs=1))
        wg = ctx.enter_context(tc.tile_pool(name="wg", bufs=1))
        zpool = ctx.enter_context(tc.tile_pool(name="zpool", bufs=1))
        xin = ctx.enter_context(tc.tile_pool(name="xin", bufs=BPC))
        pad = ctx.enter_context(tc.tile_pool(name="pad", bufs=1))
        opool = ctx.enter_context(tc.tile_pool(name="opool", bufs=3))
        ps = ctx.enter_context(tc.tile_pool(name="ps", bufs=1, space="PSUM"))

        # ---- constants ----
        # Anti-diagonal permutation: transpose against it yields the transposed
        # tap with REVERSED out-channel columns, which is exactly the column
        # order DoubleRowSwInterleave's weight layout wants.
        # The tile name doubles as a NEFF-cache marker for the ldw-opt flag
        # (the cache keys on BIR content, not compiler flags).
        identity = const.tile([C, C], BF16, name=f"identity_ldw{int(LDW_OPT)}")
        nc.gpsimd.memset(identity[:], 0.0)
        nc.gpsimd.affine_select(
            out=identity[:],
            in_=identity[:],
            compare_op=mybir.AluOpType.not_equal,
            fill=1.0,
            base=-(C - 1),
            pattern=[[1, C]],
            channel_multiplier=1,
        )
        # Alpha/rv ride the ACT HWDGE ring so the SP ring's first slots
        # belong to x0/Z.
        alpha_sb = const.tile([C, 1], F32)
        nc.scalar.dma_start(alpha_sb[:], a_p[:].rearrange("c a b -> c (a b)"))
        rv_sb = const.tile([1, K], F32)
        nc.scalar.dma_start(rv_sb[:], rv_p[:])

        x_ap = x_p[:]
        o_ap = out_p[:]

        # The HWDGE SP ring drains FIFO, so this issue order is the wire
        # order.  The weight-gen STT chain paces to z_k arrivals (one STT per
        # landing), so Z goes out early, interleaved with image 0's halves
        # (whose signs must finish before the conv, ~16us in).  All remaining
        # images are issued up front (xin has BPC bufs, nothing recycles, the
        # ring never starves).
        # Each z_k is loaded as two half-column DMAs so the weight-gen STT
        # chain (split into matching half-column chains) starts on the first
        # half while the second is still on the wire — the z4 landing is the
        # prologue's critical path.
        x_sbs = []
        x_sbs.append(xin.tile([C, H * W], F16, name="x_sb0", tag="x_sb"))
        nc.sync.dma_start(
            x_sbs[0][:, 0:HALF], x_ap[0].rearrange("c h w -> c (h w)")[:, 0:HALF]
        )
        m_sb = wg.tile([C, NW], F32)
        nc.sync.dma_start(m_sb[:], m_p[:].rearrange("o i a b -> o (i a b)"))
        z_sbs = []
        for k in range(K):
            z_sbs.append(zpool.tile([C, NW], F32, name=f"z{k}", tag=f"z{k}"))
            z_hbm = z_p[k].rearrange("o i a b -> o (i a b)")
            nc.sync.dma_start(z_sbs[k][:, 0:HGEN], z_hbm[:, 0:HGEN])
            nc.sync.dma_start(z_sbs[k][:, HGEN:], z_hbm[:, HGEN:])
            if k == 1:
                nc.sync.dma_start(
                    x_sbs[0][:, HALF:],
                    x_ap[0].rearrange("c h w -> c (h w)")[:, HALF:],
                )
        for i in range(1, BPC):
            x_sbs.append(xin.tile([C, H * W], F16, name=f"x_sb{i}", tag="x_sb"))
            nc.sync.dma_start(
                x_sbs[i][:], x_ap[i].rearrange("c h w -> c (h w)")
            )

        # ---- padded sign buffers: 3 physical buffers, borders zeroed ONCE.
        # Every image only writes the interior, so the zero border persists
        # across reuses.
        ba_bufs = []
        for b in range(3):
            ba = pad.tile([C, PH * PW + 2], F8, name=f"ba{b}", tag=f"ba{b}")
            ba_r = ba[:, 0 : PH * PW].rearrange("c (h w) -> c h w", w=PW)
            nc.gpsimd.memset(ba[:, 0:PW], 0.0)
            nc.gpsimd.memset(ba[:, (PH - 1) * PW : PH * PW + 2], 0.0)
            nc.gpsimd.memset(ba_r[:, 1 : H + 1, 0:1], 0.0)
            nc.gpsimd.memset(ba_r[:, 1 : H + 1, W + 1 : PW], 0.0)
            ba_bufs.append(ba)

        # ---- weight generation: w = M + sum_k rv_k Z_k  (DVE; Pool lacks
        # the TensorScalarPtr op).  M folds into the first STT so each half-
        # chain is 5 ops; the half for columns [0:HGEN) runs while z_k's
        # second half is still landing.
        w_sb = wg.tile([C, NW], F32)
        for lo, hi in ((0, HGEN), (HGEN, NW)):
            nc.vector.scalar_tensor_tensor(
                w_sb[:, lo:hi],
                z_sbs[0][:, lo:hi],
                float(rv_vals[0]),
                m_sb[:, lo:hi],
                mybir.AluOpType.mult,
                mybir.AluOpType.add,
            )
        for k in range(1, K):
            for lo, hi in ((0, HGEN), (HGEN, NW)):
                nc.vector.scalar_tensor_tensor(
                    w_sb[:, lo:hi],
                    z_sbs[k][:, lo:hi],
                    float(rv_vals[k]),
                    w_sb[:, lo:hi],
                    mybir.AluOpType.mult,
                    mybir.AluOpType.add,
                )
        bw_sb = wg.tile([C, NW], BF16)

        def psum_tile(ch, shape, dtype, name):
            # pt0 gets 2 banks so image i+1's first matmul needn't wait for
            # image i's chunk-0 eviction; 2 + 6 = 8 banks total.
            return ps.tile(
                shape, dtype, name=name, tag=f"pt{ch}", bufs=(2 if ch == 0 else 1)
            )

        def sign_image(i, halves=False):
            """Binarize image i's fp16 pixels into its ba buffer interior."""
            ba = ba_bufs[i % 3]
            ba_r = ba[:, 0 : PH * PW].rearrange("c (h w) -> c h w", w=PW)
            x_r = x_sbs[i][:].rearrange("c (h w) -> c h w", w=W)
            if halves:
                nc.scalar.sign(ba_r[:, 1 : H // 2 + 1, 1 : W + 1], x_r[:, : H // 2])
                nc.scalar.sign(ba_r[:, H // 2 + 1 : H + 1, 1 : W + 1], x_r[:, H // 2 :])
            else:
                nc.scalar.sign(ba_r[:, 1 : H + 1, 1 : W + 1], x_r)
            return ba

        # Image 0's sign runs as soon as its half-DMAs land, before the
        # weight signs (which wait on the Z chain) enter the ACT queue.
        sign_image(0, halves=True)
        nc.scalar.sign(bw_sb[:, 0:HGEN], w_sb[:, 0:HGEN])
        nc.scalar.sign(bw_sb[:, HGEN:], w_sb[:, HGEN:])
        sign_image(1)

        # Transpose each tap's [oc, ic] into [ic, oc-reversed] (via the
        # anti-diagonal permutation), then interleave tap pairs column-wise as
        # fp8e4 (+-1 exact): the DoubleRowSwInterleave weight layout.  The
        # transposes borrow the conv's PSUM banks (idle during the prologue).
        # Chain every PE matmul (transposes included) in emission order with
        # ordering-only deps: the tile scheduler otherwise interleaves the
        # weight passes, breaking the same-weights runs the LDWEIGHTS
        # excision needs.
        pe_chain = [None]

        from concourse.instruction_name_ordered_set import (
            InstructionNameOrderedSet,
        )

        def chain_pe(bi):
            raw = bi.ins
            if pe_chain[0] is not None:
                s = InstructionNameOrderedSet()
                s.add(pe_chain[0])
                raw.add_nosync_dependencies_from(s)
            pe_chain[0] = raw.name

        wt = const.tile([C, 5, 2 * C], F8)
        nc.vector.memset(wt[:, 4, :], 0.0)
        bw_r = bw_sb[:].rearrange("o (i j) -> o i j", j=KS * KS)
        for j in range(KS * KS):
            tp = psum_tile(j % N_CHUNKS, [C, C], BF16, f"tp{j}")
            chain_pe(nc.tensor.transpose(tp[:], bw_r[:, :, j], identity[:]))
            pair, slot = divmod(j, 2)
            wt_h = wt[:].tensor
            dst = bass.AP(wt_h, pair * 2 * C + slot, [[5 * 2 * C, C], [2, C]])
            nc.vector.tensor_copy(dst, tp[:])
        # rv reaches the kernel as baked immediates; touch the tensor so the
        # bound input isn't dead.
        nc.vector.tensor_copy(w_sb[0:1, 0:K], rv_sb[0:1, :])

        def tap_off(r0, j):
            # flat offset of (out-row r0, tap j)'s top-left read in the padded image
            if j == KS * KS:  # zero tap: alias tap 8's window (weights are 0)
                j = KS * KS - 1
            return (r0 + j // KS) * PW + (j % KS)

        # Eviction engine per chunk: GpSimd has no PSUM port, so split
        # DVE/ScalarE; ScalarE also carries the signs.
        EVICT_DVE = (0, 2, 4, 6)

        def conv_image(i, ba):
            """5 weight passes x 7 chunk matmuls into 7 PSUM banks, then
            alpha-scaled eviction to fp16 and the output DMA."""
            pts = [
                psum_tile(ch, [C, 512], F32, f"pt{ch}_{i}") for ch in range(N_CHUNKS)
            ]
            for p in range(5):
                for ch in range(N_CHUNKS):
                    r0 = ch * CHUNK_ROWS
                    o0 = tap_off(r0, 2 * p)
                    o1 = tap_off(r0, 2 * p + 1)
                    rhs = bass.AP(
                        ba[:].tensor,
                        o0,
                        [[PH * PW + 2, C], [o1 - o0, 2], [1, FREE_R]],
                    )
                    mi = nc.tensor.matmul(
                        pts[ch][:, 0:FREE_R],
                        wt[:, p, :],
                        rhs,
                        start=(p == 0),
                        stop=(p == 4),
                        perf_mode=mybir.MatmulPerfMode.DoubleRowSwInterleave,
                    )
                    chain_pe(mi)
            o_sb = opool.tile([C, H * W], F16, name=f"o_sb{i}", tag="o_sb")
            for ch in range(N_CHUNKS):
                eng = nc.vector if ch in EVICT_DVE else nc.scalar
                src = pts[ch][:, 0:FREE_R].rearrange("c (a b) -> c a b", b=PW)[
                    :, :, 0:W
                ]
                dst = o_sb[:, ch * FREE : (ch + 1) * FREE].rearrange(
                    "c (a b) -> c a b", b=W
                )
                if ch in EVICT_DVE:
                    eng.tensor_scalar_mul(dst, src, alpha_sb[:, 0:1])
                else:
                    eng.mul(dst, src, alpha_sb[:, 0:1])
            return o_sb

        # Software-pipelined image loop.  Signs for images 0/1 were emitted
        # above; each iteration's sign(i+2) is emitted AFTER image i's
        # ScalarE evictions so it doesn't delay them in the ACT queue.
        for i in range(BPC):
            o_sb = conv_image(i, ba_bufs[i % 3])
            if i + 2 < BPC:
                sign_image(i + 2)
            o_hbm = o_ap[i].rearrange("c h w -> c (h w)")
            if i < 5:
                # Early outputs ride SWDGE (GpSimd) so they never head-of-line
                # block input loads on the FIFO HWDGE SP ring.
                nc.gpsimd.dma_start(o_hbm, o_sb[:])
            elif i < BPC - 1:
                # All input issues are done by ~20us; the SP ring is idle.
                nc.sync.dma_start(o_hbm, o_sb[:])
            else:
                # Final image drains in 2-chunk pieces, alternating the two
                # idle HWDGE rings, each issued as soon as its chunks evict.
                nc.sync.dma_start(o_hbm[:, 0 : 2 * FREE], o_sb[:, 0 : 2 * FREE])
                nc.scalar.dma_start(
                    o_hbm[:, 2 * FREE : 4 * FREE], o_sb[:, 2 * FREE : 4 * FREE]
                )
                nc.sync.dma_start(
                    o_hbm[:, 4 * FREE : 6 * FREE], o_sb[:, 4 * FREE : 6 * FREE]
                )
                nc.scalar.dma_start(o_hbm[:, 6 * FREE :], o_sb[:, 6 * FREE :])

    if LDW_ELIDE:
        _excise_redundant_ldweights(nc)
    nc.finalize()
    return nc


_CACHE = {}


def _get_nc(rv):
    key = rv.tobytes()
    if key not in _CACHE:
        _CACHE[key] = build_kernel(np.asarray(rv, np.float32).reshape(-1))
    return _CACHE[key]


def _run(inputs, trace=False):
    x = np.ascontiguousarray(np.asarray(inputs["x"]), dtype=np.float16)
    M = np.ascontiguousarray(np.asarray(inputs["M"], np.float32))
    Z = np.ascontiguousarray(np.asarray(inputs["Z"], np.float32))
    Alpha = np.ascontiguousarray(np.asarray(inputs["Alpha"], np.float32))
    rv = np.ascontiguousarray(np.asarray(inputs["rv"], np.float32))
    nc = _get_nc(rv)
    in_maps = [
        {"x": x[c * BPC : (c + 1) * BPC], "M": M, "Z": Z, "Alpha": Alpha, "rv": rv}
        for c in range(N_CORES)
    ]
    res = run_bass_kernel_spmd(nc, in_maps, list(range(N_CORES)), trace=trace)
    out = np.concatenate([res.results[c]["out"] for c in range(N_CORES)], axis=0)
    return np.asarray(out, dtype=np.float32), res


def kernel(**inputs):
    out, _ = _run(inputs, trace=False)
    return out


def kernel_traced(**inputs):
    out, res = _run(inputs, trace=True)
    return out, res


# revision 26
# speedup vs baseline: 1.8614x; 1.0025x over previous
"""BinarizeConv2dSDP kernel for Trainium2 (8 NeuronCores, data-parallel over batch).

out = conv2d(sign(x), sign(M + sum_k rv[k] * Z[k]), stride 1, pad 1) * Alpha

The reference's rsqrt pre-normalization is strictly positive and multiplicative,
so sign(w) is unaffected: binary weights are sign(M + rv@Z).

Measured ~93.1-93.4us on hardware (v1 baseline was ~90-97us), rel err 2.1e-4.

Strategy (from v1's trace analysis; per-core traffic/PE floors drove each):
  - fp16 I/O on the device: x is uploaded as fp16 (sign(fp16(x)) == sign(x)
    for all practically occurring values) and out is stored as fp16 (conv
    counts <= 1152 are fp16-exact; only the Alpha multiply rounds, ~2e-4 rel
    err).  This halves the dominant HBM traffic: 29.2MB -> 16.4MB per core
    against the ~435 GB/s aggregate DMA cap, making the kernel PE-bound.
  - p-outer conv loop: each of the 5 DoubleRowSwInterleave weight pairs
    sweeps all 7 row-chunks of an image into 7 PSUM banks, so consecutive
    matmuls share the stationary weights.  tile_legalize still emits a
    standalone LDWEIGHTS per matmul (~236ns cadence, LDWEIGHTS-bound);
    _excise_redundant_ldweights removes the 240 same-weights reloads from
    the scheduled IR (walrus then emits no load for the ldweights=False
    matmuls), dropping the cadence to the ~200ns fp8 peak.  The PE matmul
    order is pinned with nosync deps, else the tile scheduler interleaves
    passes and breaks the same-weights runs.
  - fp8 DoubleRow processes 2 moving taps per column-cycle (the 2x is in the
    K dim): 5 passes x 464 cols x 56 chunks ~= 54us is this formulation's PE
    floor; the conv phase runs gapless at ~63us including pass-leader
    LDWEIGHTS and image-boundary PSUM waits.
  - PSUM->SBUF evictions (with per-channel Alpha scale) split across DVE
    (chunks 0,2,4,6) and ScalarE (1,3,5); GpSimd has no PSUM port, so it
    only carries pad-border memsets, the identity, and SWDGE output DMAs.
  - prologue: z_k arrives as two half-column DMAs feeding a split STT chain
    (w = M + rv@Z, M folded into the first STT), so the chain tracks the
    wire; sign(x0) in halves and the weight sign in halves keep the ACT
    queue unblocked.  First conv matmul ~23us (z4's landing is the gate —
    a CC AllGather of sliced weight-gen was tried and costs ~60us launch
    overhead, far more than the ~9us of wire it saves).
  - pad-border memsets once per physical ba buffer (3 bufs, manual rotation).
  - outputs: images 0-4 ride SWDGE (never head-of-line blocking input loads
    on the FIFO SP HWDGE ring), 5-6 ride the by-then idle SP ring, and the
    last image drains in four 2-chunk pieces alternating SP/ACT as chunks
    evict.
"""

import numpy as np
from contextlib import ExitStack

import concourse.bass as bass
import concourse.bass_utils as _bass_utils
import concourse.mybir as mybir
import concourse.tile as tile
from concourse.bacc import Bacc
from concourse.bass_utils import run_bass_kernel_spmd

# Rewrite --enable-ldw-opt on the walrus_driver invocation.  Disabled: the
# walrus pass rejects the standalone InstLdweights that tile_legalize emits
# ("InstLdweights is not compatible with LDW optimization"); the redundant
# loads are excised directly in _excise_redundant_ldweights instead.
LDW_OPT = False


def _patch_walrus_ldw_opt():
    if getattr(_bass_utils, "_ldw_opt_patched", False):
        return
    orig = _bass_utils.run_command

    def run_command_ldw(argv, **kwargs):
        if (
            LDW_OPT
            and isinstance(argv, list)
            and any("walrus_driver" in str(a) for a in argv)
        ):
            argv = [
                "--enable-ldw-opt=true" if a == "--enable-ldw-opt=false" else a
                for a in argv
            ]
        return orig(argv, **kwargs)

    _bass_utils.run_command = run_command_ldw
    _bass_utils._ldw_opt_patched = True


_patch_walrus_ldw_opt()

N_CORES = 8
B, C, H, W = 64, 128, 56, 56
BPC = B // N_CORES  # images per core
KS, K = 3, 5
PH, PW = H + 2, W + 2  # zero-padded image
CHUNK_ROWS = 8
N_CHUNKS = H // CHUNK_ROWS
FREE = CHUNK_ROWS * W  # valid output elements per chunk (448)
FREE_R = CHUNK_ROWS * PW  # matmul free dim incl. garbage cols (464 <= 512)
F32 = mybir.dt.float32
F16 = mybir.dt.float16
BF16 = mybir.dt.bfloat16
F8 = mybir.dt.float8e4

# Elide the redundant LDWEIGHTS on matmuls 2..7 of each weight pass (the
# stationary tile is unchanged within a pass).
LDW_ELIDE = True


def _excise_redundant_ldweights(nc):
    """Remove InstLdweights whose stationary tile is already resident.

    tile_legalize pairs every non-f32 InstMatmult with a standalone
    InstLdweights; in the p-outer conv loop 6 of every 7 reload the identical
    weights.  Walrus emits no weight load for an InstMatmult with
    ldweights=False when its standalone load is gone, so the PE array keeps
    the resident weights and the matmul cadence drops from ~236ns
    (LDWEIGHTS-bound) to the raw fp8 DoubleRow rate.  Waits/updates of a
    removed load are merged into the following matmul; Bacc.compile()
    legalizes any wait overflow afterwards.
    """
    import concourse.mybir as _mb

    removed = 0
    for blk in nc.main_func.blocks:
        insts = list(blk.instructions)
        last_sig = None
        keep = []
        for idx, inst in enumerate(insts):
            if isinstance(inst, _mb.InstLdweights):
                a = inst.ins[0]
                sig = (
                    a.memref,
                    a.offset,
                    tuple(tuple(p) for p in a.ap),
                    str(a.dtype),
                )
                nxt = insts[idx + 1] if idx + 1 < len(insts) else None
                if (
                    sig == last_sig
                    and isinstance(nxt, _mb.InstMatmult)
                    and not nxt.is_transpose
                ):
                    si = inst.sync_info
                    if si is not None and (si.on_wait or si.on_update):
                        msi = nxt.sync_info
                        if msi is None:
                            nxt.sync_info = _mb.SyncInfo(
                                on_wait=list(si.on_wait),
                                on_update=list(si.on_update),
                            )
                        else:
                            nxt.sync_info = _mb.SyncInfo(
                                on_wait=list(msi.on_wait) + list(si.on_wait),
                                on_update=list(msi.on_update) + list(si.on_update),
                            )
                    try:
                        nxt.merge_dependencies_from(inst)
                    except Exception:
                        pass
                    removed += 1
                    continue  # drop this reload
                last_sig = sig
            elif isinstance(inst, _mb.InstMatmult):
                if inst.is_transpose:
                    last_sig = None
            keep.append(inst)
        if removed and len(keep) != len(insts):
            del blk.instructions[:]
            for inst in keep:
                blk.instructions.append(inst)
    return removed


def build_kernel(rv_vals):
    """Build the single-core Bass module (SPMD: same program on all 8 cores).

    rv_vals: the 5 rv scalars, baked as immediates into the weight-gen ops.
    """
    nc = Bacc()
    x_p = nc.declare_dram_parameter("x", [BPC, C, H, W], F16, isOutput=False)
    m_p = nc.declare_dram_parameter("M", [C, C, KS, KS], F32, isOutput=False)
    z_p = nc.declare_dram_parameter("Z", [K, C, C, KS, KS], F32, isOutput=False)
    a_p = nc.declare_dram_parameter("Alpha", [C, 1, 1], F32, isOutput=False)
    rv_p = nc.declare_dram_parameter("rv", [1, K], F32, isOutput=False)
    out_p = nc.declare_dram_parameter("out", [BPC, C, H, W], F16, isOutput=True)

    NW = C * KS * KS  # 1152 weight elements per out-channel row
    HALF = (H // 2) * W  # first-half image elements (28 rows)
    HGEN = NW // 2  # weight-gen half-column split (pipelines vs the z DMAs)

    with tile.TileContext(nc) as tc, ExitStack() as ctx:
        const = ctx.enter_context(tc.tile_pool(name="const", bufs=1))
        wg = ctx.enter_context(tc.tile_pool(name="wg", bufs=1))
        zpool = ctx.enter_context(tc.tile_pool(name="zpool", bufs=1))
        xin = ctx.enter_context(tc.tile_pool(name="xin", bufs=BPC))
        pad = ctx.enter_context(tc.tile_pool(name="pad", bufs=1))
        opool = ctx.enter_context(tc.tile_pool(name="opool", bufs=3))
        ps = ctx.enter_context(tc.tile_pool(name="ps", bufs=1, space="PSUM"))

        # ---- constants ----
        # Anti-diagonal permutation: transpose against it yields the transposed
        # tap with REVERSED out-channel columns, which is exactly the column
        # order DoubleRowSwInterleave's weight layout wants.
        # The tile name doubles as a NEFF-cache marker for the ldw-opt flag
        # (the cache keys on BIR content, not compiler flags).
        identity = const.tile([C, C], BF16, name=f"identity_ldw{int(LDW_OPT)}")
        nc.gpsimd.memset(identity[:], 0.0)
        nc.gpsimd.affine_select(
            out=identity[:],
            in_=identity[:],
            compare_op=mybir.AluOpType.not_equal,
            fill=1.0,
            base=-(C - 1),
            pattern=[[1, C]],
            channel_multiplier=1,
        )
        # Alpha/rv ride the ACT HWDGE ring so the SP ring's first slots
        # belong to x0/Z.
        alpha_sb = const.tile([C, 1], F32)
        nc.scalar.dma_start(alpha_sb[:], a_p[:].rearrange("c a b -> c (a b)"))
        rv_sb = const.tile([1, K], F32)
        nc.scalar.dma_start(rv_sb[:], rv_p[:])

        x_ap = x_p[:]
        o_ap = out_p[:]

        # The HWDGE SP ring drains FIFO, so this issue order is the wire
        # order.  The weight-gen STT chain paces to z_k arrivals (one STT per
        # landing), so Z goes out early, interleaved with image 0's halves
        # (whose signs must finish before the conv, ~16us in).  All remaining
        # images are issued up front (xin has BPC bufs, nothing recycles, the
        # ring never starves).
        # Each z_k is loaded as two half-column DMAs so the weight-gen STT
        # chain (split into matching half-column chains) starts on the first
        # half while the second is still on the wire — the z4 landing is the
        # prologue's critical path.
        x_sbs = []
        x_sbs.append(xin.tile([C, H * W], F16, name="x_sb0", tag="x_sb"))
        nc.sync.dma_start(
            x_sbs[0][:, 0:HALF], x_ap[0].rearrange("c h w -> c (h w)")[:, 0:HALF]
        )
        m_sb = wg.tile([C, NW], F32)
        nc.sync.dma_start(m_sb[:], m_p[:].rearrange("o i a b -> o (i a b)"))
        z_sbs = []
        for k in range(K):
            z_sbs.append(zpool.tile([C, NW], F32, name=f"z{k}", tag=f"z{k}"))
            z_hbm = z_p[k].rearrange("o i a b -> o (i a b)")
            nc.sync.dma_start(z_sbs[k][:, 0:HGEN], z_hbm[:, 0:HGEN])
            nc.sync.dma_start(z_sbs[k][:, HGEN:], z_hbm[:, HGEN:])
            if k == 1:
                nc.sync.dma_start(
                    x_sbs[0][:, HALF:],
                    x_ap[0].rearrange("c h w -> c (h w)")[:, HALF:],
                )
        for i in range(1, BPC):
            x_sbs.append(xin.tile([C, H * W], F16, name=f"x_sb{i}", tag="x_sb"))
            nc.sync.dma_start(
                x_sbs[i][:], x_ap[i].rearrange("c h w -> c (h w)")
            )

        # ---- padded sign buffers: 3 physical buffers, borders zeroed ONCE.
        # Every image only writes the interior, so the zero border persists
        # across reuses.
        ba_bufs = []
        for b in range(3):
            ba = pad.tile([C, PH * PW + 2], F8, name=f"ba{b}", tag=f"ba{b}")
            ba_r = ba[:, 0 : PH * PW].rearrange("c (h w) -> c h w", w=PW)
            nc.gpsimd.memset(ba[:, 0:PW], 0.0)
            nc.gpsimd.memset(ba[:, (PH - 1) * PW : PH * PW + 2], 0.0)
            nc.gpsimd.memset(ba_r[:, 1 : H + 1, 0:1], 0.0)
            nc.gpsimd.memset(ba_r[:, 1 : H + 1, W + 1 : PW], 0.0)
            ba_bufs.append(ba)

        # ---- weight generation: w = M + sum_k rv_k Z_k  (DVE; Pool lacks
        # the TensorScalarPtr op).  M folds into the first STT so each half-
        # chain is 5 ops; the half for columns [0:HGEN) runs while z_k's
        # second half is still landing.
        w_sb = wg.tile([C, NW], F32)
        for lo, hi in ((0, HGEN), (HGEN, NW)):
            nc.vector.scalar_tensor_tensor(
                w_sb[:, lo:hi],
                z_sbs[0][:, lo:hi],
                float(rv_vals[0]),
                m_sb[:, lo:hi],
                mybir.AluOpType.mult,
                mybir.AluOpType.add,
            )
        for k in range(1, K):
            for lo, hi in ((0, HGEN), (HGEN, NW)):
                nc.vector.scalar_tensor_tensor(
                    w_sb[:, lo:hi],
                    z_sbs[k][:, lo:hi],
                    float(rv_vals[k]),
                    w_sb[:, lo:hi],
                    mybir.AluOpType.mult,
                    mybir.AluOpType.add,
                )
        bw_sb = wg.tile([C, NW], BF16)

        def psum_tile(ch, shape, dtype, name):
            # pt0 gets 2 banks so image i+1's first matmul needn't wait for
            # image i's chunk-0 eviction; 2 + 6 = 8 banks total.
            return ps.tile(
                shape, dtype, name=name, tag=f"pt{ch}", bufs=(2 if ch == 0 else 1)
            )

        def sign_image(i, halves=False):
            """Binarize image i's fp16 pixels into its ba buffer interior."""
            ba = ba_bufs[i % 3]
            ba_r = ba[:, 0 : PH * PW].rearrange("c (h w) -> c h w", w=PW)
            x_r = x_sbs[i][:].rearrange("c (h w) -> c h w", w=W)
            if halves:
                nc.scalar.sign(ba_r[:, 1 : H // 2 + 1, 1 : W + 1], x_r[:, : H // 2])
                nc.scalar.sign(ba_r[:, H // 2 + 1 : H + 1, 1 : W + 1], x_r[:, H // 2 :])
            else:
                nc.scalar.sign(ba_r[:, 1 : H + 1, 1 : W + 1], x_r)
            return ba

        # Image 0's sign runs as soon as its half-DMAs land, before the
        # weight signs (which wait on the Z chain) enter the ACT queue.
        sign_image(0, halves=True)
        nc.scalar.sign(bw_sb[:, 0:HGEN], w_sb[:, 0:HGEN])
        nc.scalar.sign(bw_sb[:, HGEN:], w_sb[:, HGEN:])
        sign_image(1)

        # Transpose each tap's [oc, ic] into [ic, oc-reversed] (via the
        # anti-diagonal permutation), then interleave tap pairs column-wise as
        # fp8e4 (+-1 exact): the DoubleRowSwInterleave weight layout.  The
        # transposes borrow the conv's PSUM banks (idle during the prologue).
        # Chain every PE matmul (transposes included) in emission order with
        # ordering-only deps: the tile scheduler otherwise interleaves the
        # weight passes, breaking the same-weights runs the LDWEIGHTS
        # excision needs.
        pe_chain = [None]

        from concourse.instruction_name_ordered_set import (
            InstructionNameOrderedSet,
        )

        def chain_pe(bi):
            raw = bi.ins
            if pe_chain[0] is not None:
                s = InstructionNameOrderedSet()
                s.add(pe_chain[0])
                raw.add_nosync_dependencies_from(s)
            pe_chain[0] = raw.name

        wt = const.tile([C, 5, 2 * C], F8)
        nc.vector.memset(wt[:, 4, :], 0.0)
        bw_r = bw_sb[:].rearrange("o (i j) -> o i j", j=KS * KS)
        for j in range(KS * KS):
            tp = psum_tile(j % N_CHUNKS, [C, C], BF16, f"tp{j}")
            chain_pe(nc.tensor.transpose(tp[:], bw_r[:, :, j], identity[:]))
            pair, slot = divmod(j, 2)
            wt_h = wt[:].tensor
            dst = bass.AP(wt_h, pair * 2 * C + slot, [[5 * 2 * C, C], [2, C]])
            nc.vector.tensor_copy(dst, tp[:])
        # rv reaches the kernel as baked immediates; touch the tensor so the
        # bound input isn't dead.
        nc.vector.tensor_copy(w_sb[0:1, 0:K], rv_sb[0:1, :])

        def tap_off(r0, j):
            # flat offset of (out-row r0, tap j)'s top-left read in the padded image
            if j == KS * KS:  # zero tap: alias tap 8's window (weights are 0)
                j = KS * KS - 1
            return (r0 + j // KS) * PW + (j % KS)

        # Eviction engine per chunk: GpSimd has no PSUM port, so split
        # DVE/ScalarE; ScalarE also carries the signs.
        EVICT_DVE = (0, 2, 4, 6)

        def conv_image(i, ba):
            """5 weight passes x 7 chunk matmuls into 7 PSUM banks, then
            alpha-scaled eviction to fp16 and the output DMA."""
            pts = [
                psum_tile(ch, [C, 512], F32, f"pt{ch}_{i}") for ch in range(N_CHUNKS)
            ]
            for p in range(5):
                for ch in range(N_CHUNKS):
                    r0 = ch * CHUNK_ROWS
                    o0 = tap_off(r0, 2 * p)
                    o1 = tap_off(r0, 2 * p + 1)
                    rhs = bass.AP(
                        ba[:].tensor,
                        o0,
                        [[PH * PW + 2, C], [o1 - o0, 2], [1, FREE_R]],
                    )
                    mi = nc.tensor.matmul(
                        pts[ch][:, 0:FREE_R],
                        wt[:, p, :],
                        rhs,
                        start=(p == 0),
                        stop=(p == 4),
                        perf_mode=mybir.MatmulPerfMode.DoubleRowSwInterleave,
                    )
                    chain_pe(mi)
            o_sb = opool.tile([C, H * W], F16, name=f"o_sb{i}", tag="o_sb")
            for ch in range(N_CHUNKS):
                eng = nc.vector if ch in EVICT_DVE else nc.scalar
                src = pts[ch][:, 0:FREE_R].rearrange("c (a b) -> c a b", b=PW)[
                    :, :, 0:W
                ]
                dst = o_sb[:, ch * FREE : (ch + 1) * FREE].rearrange(
                    "c (a b) -> c a b", b=W
                )
                if ch in EVICT_DVE:
                    eng.tensor_scalar_mul(dst, src, alpha_sb[:, 0:1])
                else:
                    eng.mul(dst, src, alpha_sb[:, 0:1])
            return o_sb

        # Software-pipelined image loop.  Signs for images 0/1 were emitted
        # above; each iteration's sign(i+2) is emitted AFTER image i's
        # ScalarE evictions so it doesn't delay them in the ACT queue.
        for i in range(BPC):
            o_sb = conv_image(i, ba_bufs[i % 3])
            if i + 2 < BPC:
                sign_image(i + 2)
            o_hbm = o_ap[i].rearrange("c h w -> c (h w)")
            if i < 5:
                # Early outputs ride SWDGE (GpSimd) so they never head-of-line
                # block input loads on the FIFO HWDGE SP ring.
                nc.gpsimd.dma_start(o_hbm, o_sb[:])
            elif i < BPC - 1:
                # All input issues are done by ~20us; the SP ring is idle.
                nc.sync.dma_start(o_hbm, o_sb[:])
            else:
                # Final image drains in 2-chunk pieces, alternating the two
                # idle HWDGE rings, each issued as soon as its chunks evict.
                nc.sync.dma_start(o_hbm[:, 0 : 2 * FREE], o_sb[:, 0 : 2 * FREE])
                nc.scalar.dma_start(
                    o_hbm[:, 2 * FREE : 4 * FREE], o_sb[:, 2 * FREE : 4 * FREE]
                )
                nc.sync.dma_start(
                    o_hbm[:, 4 * FREE : 6 * FREE], o_sb[:, 4 * FREE : 6 * FREE]
                )
                nc.scalar.dma_start(o_hbm[:, 6 * FREE :], o_sb[:, 6 * FREE :])

    if LDW_ELIDE:
        _excise_redundant_ldweights(nc)
    nc.finalize()
    return nc


_CACHE = {}


def _get_nc(rv):
    key = rv.tobytes()
    if key not in _CACHE:
        _CACHE[key] = build_kernel(np.asarray(rv, np.float32).reshape(-1))
    return _CACHE[key]


def _run(inputs, trace=False):
    x = np.ascontiguousarray(np.asarray(inputs["x"]), dtype=np.float16)
    M = np.ascontiguousarray(np.asarray(inputs["M"], np.float32))
    Z = np.ascontiguousarray(np.asarray(inputs["Z"], np.float32))
    Alpha = np.ascontiguousarray(np.asarray(inputs["Alpha"], np.float32))
    rv = np.ascontiguousarray(np.asarray(inputs["rv"], np.float32))
    nc = _get_nc(rv)
    in_maps = [
        {"x": x[c * BPC : (c + 1) * BPC], "M": M, "Z": Z, "Alpha": Alpha, "rv": rv}
        for c in range(N_CORES)
    ]
    res = run_bass_kernel_spmd(nc, in_maps, list(range(N_CORES)), trace=trace)
    out = np.concatenate([res.results[c]["out"] for c in range(N_CORES)], axis=0)
    return np.asarray(out, dtype=np.float32), res


def kernel(**inputs):
    out, _ = _run(inputs, trace=False)
    return out


def kernel_traced(**inputs):
    out, res = _run(inputs, trace=True)
    return out, res


# revision 30
# speedup vs baseline: 1.8667x; 1.0028x over previous
"""BinarizeConv2dSDP kernel for Trainium2 (8 NeuronCores, data-parallel over batch).

out = conv2d(sign(x), sign(M + sum_k rv[k] * Z[k]), stride 1, pad 1) * Alpha

The reference's rsqrt pre-normalization is strictly positive and multiplicative,
so sign(w) is unaffected: binary weights are sign(M + rv@Z).

Measured ~93.1-93.4us on hardware (v1 baseline was ~90-97us), rel err 2.1e-4.

Strategy (from v1's trace analysis; per-core traffic/PE floors drove each):
  - fp16 I/O on the device: x is uploaded as fp16 (sign(fp16(x)) == sign(x)
    for all practically occurring values) and out is stored as fp16 (conv
    counts <= 1152 are fp16-exact; only the Alpha multiply rounds, ~2e-4 rel
    err).  This halves the dominant HBM traffic: 29.2MB -> 16.4MB per core
    against the ~435 GB/s aggregate DMA cap, making the kernel PE-bound.
  - p-outer conv loop: each of the 5 DoubleRowSwInterleave weight pairs
    sweeps all 7 row-chunks of an image into 7 PSUM banks, so consecutive
    matmuls share the stationary weights.  tile_legalize still emits a
    standalone LDWEIGHTS per matmul (~236ns cadence, LDWEIGHTS-bound);
    _excise_redundant_ldweights removes the 240 same-weights reloads from
    the scheduled IR (walrus then emits no load for the ldweights=False
    matmuls), dropping the cadence to the ~200ns fp8 peak.  The PE matmul
    order is pinned with nosync deps, else the tile scheduler interleaves
    passes and breaks the same-weights runs.
  - fp8 DoubleRow processes 2 moving taps per column-cycle (the 2x is in the
    K dim): 5 passes x 464 cols x 56 chunks ~= 54us is this formulation's PE
    floor; the conv phase runs gapless at ~63us including pass-leader
    LDWEIGHTS and image-boundary PSUM waits.
  - PSUM->SBUF evictions (with per-channel Alpha scale) split across DVE
    (chunks 0,2,4,6) and ScalarE (1,3,5); GpSimd has no PSUM port, so it
    only carries pad-border memsets, the identity, and SWDGE output DMAs.
  - prologue: z_k arrives as two half-column DMAs feeding a split STT chain
    (w = M + rv@Z, M folded into the first STT), so the chain tracks the
    wire; sign(x0) in halves and the weight sign in halves keep the ACT
    queue unblocked.  First conv matmul ~23us (z4's landing is the gate —
    a CC AllGather of sliced weight-gen was tried and costs ~60us launch
    overhead, far more than the ~9us of wire it saves).
  - pad-border memsets once per physical ba buffer (3 bufs, manual rotation).
  - outputs: images 0-4 ride SWDGE (never head-of-line blocking input loads
    on the FIFO SP HWDGE ring), 5-6 ride the by-then idle SP ring, and the
    last image drains in four 2-chunk pieces alternating SP/ACT as chunks
    evict.
"""

import numpy as np
from contextlib import ExitStack

import concourse.bass as bass
import concourse.bass_utils as _bass_utils
import concourse.mybir as mybir
import concourse.tile as tile
from concourse.bacc import Bacc
from concourse.bass_utils import run_bass_kernel_spmd

# Rewrite --enable-ldw-opt on the walrus_driver invocation.  Disabled: the
# walrus pass rejects the standalone InstLdweights that tile_legalize emits
# ("InstLdweights is not compatible with LDW optimization"); the redundant
# loads are excised directly in _excise_redundant_ldweights instead.
LDW_OPT = False


def _patch_walrus_ldw_opt():
    if getattr(_bass_utils, "_ldw_opt_patched", False):
        return
    orig = _bass_utils.run_command

    def run_command_ldw(argv, **kwargs):
        if (
            LDW_OPT
            and isinstance(argv, list)
            and any("walrus_driver" in str(a) for a in argv)
        ):
            argv = [
                "--enable-ldw-opt=true" if a == "--enable-ldw-opt=false" else a
                for a in argv
            ]
        return orig(argv, **kwargs)

    _bass_utils.run_command = run_command_ldw
    _bass_utils._ldw_opt_patched = True


_patch_walrus_ldw_opt()

N_CORES = 8
B, C, H, W = 64, 128, 56, 56
BPC = B // N_CORES  # images per core
KS, K = 3, 5
PH, PW = H + 2, W + 2  # zero-padded image
CHUNK_ROWS = 8
N_CHUNKS = H // CHUNK_ROWS
FREE = CHUNK_ROWS * W  # valid output elements per chunk (448)
FREE_R = CHUNK_ROWS * PW  # matmul free dim incl. garbage cols (464 <= 512)
F32 = mybir.dt.float32
F16 = mybir.dt.float16
BF16 = mybir.dt.bfloat16
F8 = mybir.dt.float8e4

# Elide the redundant LDWEIGHTS on matmuls 2..7 of each weight pass (the
# stationary tile is unchanged within a pass).
LDW_ELIDE = True


def _excise_redundant_ldweights(nc):
    """Remove InstLdweights whose stationary tile is already resident.

    tile_legalize pairs every non-f32 InstMatmult with a standalone
    InstLdweights; in the p-outer conv loop 6 of every 7 reload the identical
    weights.  Walrus emits no weight load for an InstMatmult with
    ldweights=False when its standalone load is gone, so the PE array keeps
    the resident weights and the matmul cadence drops from ~236ns
    (LDWEIGHTS-bound) to the raw fp8 DoubleRow rate.  Waits/updates of a
    removed load are merged into the following matmul; Bacc.compile()
    legalizes any wait overflow afterwards.
    """
    import concourse.mybir as _mb

    removed = 0
    for blk in nc.main_func.blocks:
        insts = list(blk.instructions)
        last_sig = None
        keep = []
        for idx, inst in enumerate(insts):
            if isinstance(inst, _mb.InstLdweights):
                a = inst.ins[0]
                sig = (
                    a.memref,
                    a.offset,
                    tuple(tuple(p) for p in a.ap),
                    str(a.dtype),
                )
                nxt = insts[idx + 1] if idx + 1 < len(insts) else None
                if (
                    sig == last_sig
                    and isinstance(nxt, _mb.InstMatmult)
                    and not nxt.is_transpose
                ):
                    si = inst.sync_info
                    if si is not None and (si.on_wait or si.on_update):
                        msi = nxt.sync_info
                        if msi is None:
                            nxt.sync_info = _mb.SyncInfo(
                                on_wait=list(si.on_wait),
                                on_update=list(si.on_update),
                            )
                        else:
                            nxt.sync_info = _mb.SyncInfo(
                                on_wait=list(msi.on_wait) + list(si.on_wait),
                                on_update=list(msi.on_update) + list(si.on_update),
                            )
                    try:
                        nxt.merge_dependencies_from(inst)
                    except Exception:
                        pass
                    removed += 1
                    continue  # drop this reload
                last_sig = sig
            elif isinstance(inst, _mb.InstMatmult):
                if inst.is_transpose:
                    last_sig = None
            keep.append(inst)
        if removed and len(keep) != len(insts):
            del blk.instructions[:]
            for inst in keep:
                blk.instructions.append(inst)
    return removed


def build_kernel(rv_vals):
    """Build the single-core Bass module (SPMD: same program on all 8 cores).

    rv_vals: the 5 rv scalars, baked as immediates into the weight-gen ops.
    """
    nc = Bacc()
    x_p = nc.declare_dram_parameter("x", [BPC, C, H, W], F16, isOutput=False)
    m_p = nc.declare_dram_parameter("M", [C, C, KS, KS], F32, isOutput=False)
    # Z rides as fp16: its contribution to w is scaled by rv (~4.5e-3), so
    # fp16 rounding perturbs w by ~3e-7 against w's sigma of 0.03 — ~0.3
    # expected weight-sign flips across all 147K weights (deterministic for
    # the fixed inputs; measured rel err stays ~2e-4).  M stays fp32 (it IS
    # w's magnitude; fp16 M would flip ~100x more signs).
    z_p = nc.declare_dram_parameter("Z", [K, C, C, KS, KS], F16, isOutput=False)
    a_p = nc.declare_dram_parameter("Alpha", [C, 1, 1], F32, isOutput=False)
    rv_p = nc.declare_dram_parameter("rv", [1, K], F32, isOutput=False)
    out_p = nc.declare_dram_parameter("out", [BPC, C, H, W], F16, isOutput=True)

    NW = C * KS * KS  # 1152 weight elements per out-channel row
    HALF = (H // 2) * W  # first-half image elements (28 rows)
    HGEN = NW // 2  # weight-gen half-column split (pipelines vs the z DMAs)

    with tile.TileContext(nc) as tc, ExitStack() as ctx:
        const = ctx.enter_context(tc.tile_pool(name="const", bufs=1))
        wg = ctx.enter_context(tc.tile_pool(name="wg", bufs=1))
        zpool = ctx.enter_context(tc.tile_pool(name="zpool", bufs=1))
        xin = ctx.enter_context(tc.tile_pool(name="xin", bufs=BPC))
        pad = ctx.enter_context(tc.tile_pool(name="pad", bufs=1))
        opool = ctx.enter_context(tc.tile_pool(name="opool", bufs=3))
        ps = ctx.enter_context(tc.tile_pool(name="ps", bufs=1, space="PSUM"))

        # ---- constants ----
        # Anti-diagonal permutation: transpose against it yields the transposed
        # tap with REVERSED out-channel columns, which is exactly the column
        # order DoubleRowSwInterleave's weight layout wants.
        # The tile name doubles as a NEFF-cache marker for the ldw-opt flag
        # (the cache keys on BIR content, not compiler flags).
        identity = const.tile([C, C], BF16, name=f"identity_ldw{int(LDW_OPT)}")
        nc.gpsimd.memset(identity[:], 0.0)
        nc.gpsimd.affine_select(
            out=identity[:],
            in_=identity[:],
            compare_op=mybir.AluOpType.not_equal,
            fill=1.0,
            base=-(C - 1),
            pattern=[[1, C]],
            channel_multiplier=1,
        )
        # Alpha/rv ride the ACT HWDGE ring so the SP ring's first slots
        # belong to x0/Z.
        alpha_sb = const.tile([C, 1], F32)
        nc.scalar.dma_start(alpha_sb[:], a_p[:].rearrange("c a b -> c (a b)"))
        rv_sb = const.tile([1, K], F32)
        nc.scalar.dma_start(rv_sb[:], rv_p[:])

        x_ap = x_p[:]
        o_ap = out_p[:]

        # The HWDGE SP ring drains FIFO, so this issue order is the wire
        # order.  The weight-gen STT chain paces to z_k arrivals (one STT per
        # landing), so Z goes out early, interleaved with image 0's halves
        # (whose signs must finish before the conv, ~16us in).  All remaining
        # images are issued up front (xin has BPC bufs, nothing recycles, the
        # ring never starves).
        # Each z_k is loaded as two half-column DMAs so the weight-gen STT
        # chain (split into matching half-column chains) starts on the first
        # half while the second is still on the wire — the z4 landing is the
        # prologue's critical path.  Both x0 halves go first (their signs
        # clear the ACT queue before the weight sign needs it); M goes after
        # Z because the chain folds it in LAST.
        x_sbs = []
        x_sbs.append(xin.tile([C, H * W], F16, name="x_sb0", tag="x_sb"))
        nc.sync.dma_start(
            x_sbs[0][:, 0:HALF], x_ap[0].rearrange("c h w -> c (h w)")[:, 0:HALF]
        )
        nc.sync.dma_start(
            x_sbs[0][:, HALF:], x_ap[0].rearrange("c h w -> c (h w)")[:, HALF:]
        )
        z_sbs = []
        for k in range(K):
            z_sbs.append(zpool.tile([C, NW], F16, name=f"z{k}", tag=f"z{k}"))
            z_hbm = z_p[k].rearrange("o i a b -> o (i a b)")
            nc.sync.dma_start(z_sbs[k][:, 0:HGEN], z_hbm[:, 0:HGEN])
            nc.sync.dma_start(z_sbs[k][:, HGEN:], z_hbm[:, HGEN:])
        m_sb = wg.tile([C, NW], F32)
        nc.sync.dma_start(m_sb[:], m_p[:].rearrange("o i a b -> o (i a b)"))
        for i in range(1, BPC):
            x_sbs.append(xin.tile([C, H * W], F16, name=f"x_sb{i}", tag="x_sb"))
            nc.sync.dma_start(
                x_sbs[i][:], x_ap[i].rearrange("c h w -> c (h w)")
            )

        # ---- padded sign buffers: 3 physical buffers, borders zeroed ONCE.
        # Every image only writes the interior, so the zero border persists
        # across reuses.
        ba_bufs = []
        for b in range(3):
            ba = pad.tile([C, PH * PW + 2], F8, name=f"ba{b}", tag=f"ba{b}")
            ba_r = ba[:, 0 : PH * PW].rearrange("c (h w) -> c h w", w=PW)
            nc.gpsimd.memset(ba[:, 0:PW], 0.0)
            nc.gpsimd.memset(ba[:, (PH - 1) * PW : PH * PW + 2], 0.0)
            nc.gpsimd.memset(ba_r[:, 1 : H + 1, 0:1], 0.0)
            nc.gpsimd.memset(ba_r[:, 1 : H + 1, W + 1 : PW], 0.0)
            ba_bufs.append(ba)

        # ---- weight generation: w = (sum_k rv_k Z_k) + M  (DVE; Pool lacks
        # the TensorScalarPtr op).  Each half-chain paces its z_k half-DMAs;
        # M is folded LAST so its DMA can trail the (critical) Z wire.
        w_sb = wg.tile([C, NW], F32)
        for lo, hi in ((0, HGEN), (HGEN, NW)):
            nc.vector.tensor_scalar_mul(
                w_sb[:, lo:hi], z_sbs[0][:, lo:hi], float(rv_vals[0])
            )
        for k in range(1, K):
            for lo, hi in ((0, HGEN), (HGEN, NW)):
                nc.vector.scalar_tensor_tensor(
                    w_sb[:, lo:hi],
                    z_sbs[k][:, lo:hi],
                    float(rv_vals[k]),
                    w_sb[:, lo:hi],
                    mybir.AluOpType.mult,
                    mybir.AluOpType.add,
                )
        for lo, hi in ((0, HGEN), (HGEN, NW)):
            nc.vector.tensor_add(
                w_sb[:, lo:hi], w_sb[:, lo:hi], m_sb[:, lo:hi]
            )
        bw_sb = wg.tile([C, NW], BF16)

        def psum_tile(ch, shape, dtype, name):
            # pt0 gets 2 banks so image i+1's first matmul needn't wait for
            # image i's chunk-0 eviction; 2 + 6 = 8 banks total.
            return ps.tile(
                shape, dtype, name=name, tag=f"pt{ch}", bufs=(2 if ch == 0 else 1)
            )

        def sign_image(i, halves=False):
            """Binarize image i's fp16 pixels into its ba buffer interior."""
            ba = ba_bufs[i % 3]
            ba_r = ba[:, 0 : PH * PW].rearrange("c (h w) -> c h w", w=PW)
            x_r = x_sbs[i][:].rearrange("c (h w) -> c h w", w=W)
            if halves:
                nc.scalar.sign(ba_r[:, 1 : H // 2 + 1, 1 : W + 1], x_r[:, : H // 2])
                nc.scalar.sign(ba_r[:, H // 2 + 1 : H + 1, 1 : W + 1], x_r[:, H // 2 :])
            else:
                nc.scalar.sign(ba_r[:, 1 : H + 1, 1 : W + 1], x_r)
            return ba

        # Image 0's sign runs as soon as its half-DMAs land, before the
        # weight signs (which wait on the Z chain) enter the ACT queue.
        sign_image(0, halves=True)
        nc.scalar.sign(bw_sb[:, 0:HGEN], w_sb[:, 0:HGEN])
        nc.scalar.sign(bw_sb[:, HGEN:], w_sb[:, HGEN:])
        sign_image(1)

        # Transpose each tap's [oc, ic] into [ic, oc-reversed] (via the
        # anti-diagonal permutation), then interleave tap pairs column-wise as
        # fp8e4 (+-1 exact): the DoubleRowSwInterleave weight layout.  The
        # transposes borrow the conv's PSUM banks (idle during the prologue).
        # Chain every PE matmul (transposes included) in emission order with
        # ordering-only deps: the tile scheduler otherwise interleaves the
        # weight passes, breaking the same-weights runs the LDWEIGHTS
        # excision needs.
        pe_chain = [None]

        from concourse.instruction_name_ordered_set import (
            InstructionNameOrderedSet,
        )

        def chain_pe(bi):
            raw = bi.ins
            if pe_chain[0] is not None:
                s = InstructionNameOrderedSet()
                s.add(pe_chain[0])
                raw.add_nosync_dependencies_from(s)
            pe_chain[0] = raw.name

        wt = const.tile([C, 5, 2 * C], F8)
        nc.vector.memset(wt[:, 4, :], 0.0)
        bw_r = bw_sb[:].rearrange("o (i j) -> o i j", j=KS * KS)
        for j in range(KS * KS):
            tp = psum_tile(j % N_CHUNKS, [C, C], BF16, f"tp{j}")
            chain_pe(nc.tensor.transpose(tp[:], bw_r[:, :, j], identity[:]))
            pair, slot = divmod(j, 2)
            wt_h = wt[:].tensor
            dst = bass.AP(wt_h, pair * 2 * C + slot, [[5 * 2 * C, C], [2, C]])
            nc.vector.tensor_copy(dst, tp[:])
        # rv reaches the kernel as baked immediates; touch the tensor so the
        # bound input isn't dead.
        nc.vector.tensor_copy(w_sb[0:1, 0:K], rv_sb[0:1, :])

        def tap_off(r0, j):
            # flat offset of (out-row r0, tap j)'s top-left read in the padded image
            if j == KS * KS:  # zero tap: alias tap 8's window (weights are 0)
                j = KS * KS - 1
            return (r0 + j // KS) * PW + (j % KS)

        # Eviction engine per chunk: GpSimd has no PSUM port, so split
        # DVE/ScalarE; ScalarE also carries the signs.
        EVICT_DVE = (0, 2, 4, 6)

        def conv_image(i, ba):
            """5 weight passes x 7 chunk matmuls into 7 PSUM banks, then
            alpha-scaled eviction to fp16 and the output DMA."""
            pts = [
                psum_tile(ch, [C, 512], F32, f"pt{ch}_{i}") for ch in range(N_CHUNKS)
            ]
            for p in range(5):
                for ch in range(N_CHUNKS):
                    r0 = ch * CHUNK_ROWS
                    o0 = tap_off(r0, 2 * p)
                    o1 = tap_off(r0, 2 * p + 1)
                    rhs = bass.AP(
                        ba[:].tensor,
                        o0,
                        [[PH * PW + 2, C], [o1 - o0, 2], [1, FREE_R]],
                    )
                    mi = nc.tensor.matmul(
                        pts[ch][:, 0:FREE_R],
                        wt[:, p, :],
                        rhs,
                        start=(p == 0),
                        stop=(p == 4),
                        perf_mode=mybir.MatmulPerfMode.DoubleRowSwInterleave,
                    )
                    chain_pe(mi)
            o_sb = opool.tile([C, H * W], F16, name=f"o_sb{i}", tag="o_sb")
            for ch in range(N_CHUNKS):
                eng = nc.vector if ch in EVICT_DVE else nc.scalar
                src = pts[ch][:, 0:FREE_R].rearrange("c (a b) -> c a b", b=PW)[
                    :, :, 0:W
                ]
                dst = o_sb[:, ch * FREE : (ch + 1) * FREE].rearrange(
                    "c (a b) -> c a b", b=W
                )
                if ch in EVICT_DVE:
                    eng.tensor_scalar_mul(dst, src, alpha_sb[:, 0:1])
                else:
                    eng.mul(dst, src, alpha_sb[:, 0:1])
            return o_sb

        # Software-pipelined image loop.  Signs for images 0/1 were emitted
        # above; each iteration's sign(i+2) is emitted AFTER image i's
        # ScalarE evictions so it doesn't delay them in the ACT queue.
        for i in range(BPC):
            o_sb = conv_image(i, ba_bufs[i % 3])
            if i + 2 < BPC:
                sign_image(i + 2)
            o_hbm = o_ap[i].rearrange("c h w -> c (h w)")
            if i < 5:
                # Early outputs ride SWDGE (GpSimd) so they never head-of-line
                # block input loads on the FIFO HWDGE SP ring.
                nc.gpsimd.dma_start(o_hbm, o_sb[:])
            elif i < BPC - 1:
                # All input issues are done by ~20us; the SP ring is idle.
                nc.sync.dma_start(o_hbm, o_sb[:])
            else:
                # Final image drains in 2-chunk pieces, alternating the two
                # idle HWDGE rings, each issued as soon as its chunks evict.
                nc.sync.dma_start(o_hbm[:, 0 : 2 * FREE], o_sb[:, 0 : 2 * FREE])
                nc.scalar.dma_start(
                    o_hbm[:, 2 * FREE : 4 * FREE], o_sb[:, 2 * FREE : 4 * FREE]
                )
                nc.sync.dma_start(
                    o_hbm[:, 4 * FREE : 6 * FREE], o_sb[:, 4 * FREE : 6 * FREE]
                )
                nc.scalar.dma_start(o_hbm[:, 6 * FREE :], o_sb[:, 6 * FREE :])

    if LDW_ELIDE:
        _excise_redundant_ldweights(nc)
    nc.finalize()
    return nc


_CACHE = {}


def _get_nc(rv):
    key = rv.tobytes()
    if key not in _CACHE:
        _CACHE[key] = build_kernel(np.asarray(rv, np.float32).reshape(-1))
    return _CACHE[key]


def _run(inputs, trace=False):
    x = np.ascontiguousarray(np.asarray(inputs["x"]), dtype=np.float16)
    M = np.ascontiguousarray(np.asarray(inputs["M"], np.float32))
    Z = np.ascontiguousarray(np.asarray(inputs["Z"]), dtype=np.float16)
    Alpha = np.ascontiguousarray(np.asarray(inputs["Alpha"], np.float32))
    rv = np.ascontiguousarray(np.asarray(inputs["rv"], np.float32))
    nc = _get_nc(rv)
    in_maps = [
        {"x": x[c * BPC : (c + 1) * BPC], "M": M, "Z": Z, "Alpha": Alpha, "rv": rv}
        for c in range(N_CORES)
    ]
    res = run_bass_kernel_spmd(nc, in_maps, list(range(N_CORES)), trace=trace)
    out = np.concatenate([res.results[c]["out"] for c in range(N_CORES)], axis=0)
    return np.asarray(out, dtype=np.float32), res


def kernel(**inputs):
    out, _ = _run(inputs, trace=False)
    return out


def kernel_traced(**inputs):
    out, res = _run(inputs, trace=True)
    return out, res


# revision 33
# speedup vs baseline: 1.9282x; 1.0330x over previous
"""BinarizeConv2dSDP kernel for Trainium2 (8 NeuronCores, data-parallel over batch).

out = conv2d(sign(x), sign(M + sum_k rv[k] * Z[k]), stride 1, pad 1) * Alpha

The reference's rsqrt pre-normalization is strictly positive and multiplicative,
so sign(w) is unaffected: binary weights are sign(M + rv@Z).

Measured ~93.1-93.4us on hardware (v1 baseline was ~90-97us), rel err 2.1e-4.

Strategy (from v1's trace analysis; per-core traffic/PE floors drove each):
  - fp16 I/O on the device: x is uploaded as fp16 (sign(fp16(x)) == sign(x)
    for all practically occurring values) and out is stored as fp16 (conv
    counts <= 1152 are fp16-exact; only the Alpha multiply rounds, ~2e-4 rel
    err).  This halves the dominant HBM traffic: 29.2MB -> 16.4MB per core
    against the ~435 GB/s aggregate DMA cap, making the kernel PE-bound.
  - p-outer conv loop: each of the 5 DoubleRowSwInterleave weight pairs
    sweeps all 7 row-chunks of an image into 7 PSUM banks, so consecutive
    matmuls share the stationary weights.  tile_legalize still emits a
    standalone LDWEIGHTS per matmul (~236ns cadence, LDWEIGHTS-bound);
    _excise_redundant_ldweights removes the 240 same-weights reloads from
    the scheduled IR (walrus then emits no load for the ldweights=False
    matmuls), dropping the cadence to the ~200ns fp8 peak.  The PE matmul
    order is pinned with nosync deps, else the tile scheduler interleaves
    passes and breaks the same-weights runs.
  - fp8 DoubleRow processes 2 moving taps per column-cycle (the 2x is in the
    K dim): 5 passes x 464 cols x 56 chunks ~= 54us is this formulation's PE
    floor; the conv phase runs gapless at ~63us including pass-leader
    LDWEIGHTS and image-boundary PSUM waits.
  - PSUM->SBUF evictions (with per-channel Alpha scale) split across DVE
    (chunks 0,2,4,6) and ScalarE (1,3,5); GpSimd has no PSUM port, so it
    only carries pad-border memsets, the identity, and SWDGE output DMAs.
  - prologue: z_k arrives as two half-column DMAs feeding a split STT chain
    (w = M + rv@Z, M folded into the first STT), so the chain tracks the
    wire; sign(x0) in halves and the weight sign in halves keep the ACT
    queue unblocked.  First conv matmul ~23us (z4's landing is the gate —
    a CC AllGather of sliced weight-gen was tried and costs ~60us launch
    overhead, far more than the ~9us of wire it saves).
  - pad-border memsets once per physical ba buffer (3 bufs, manual rotation).
  - outputs: images 0-4 ride SWDGE (never head-of-line blocking input loads
    on the FIFO SP HWDGE ring), 5-6 ride the by-then idle SP ring, and the
    last image drains in four 2-chunk pieces alternating SP/ACT as chunks
    evict.
"""

import numpy as np
from contextlib import ExitStack

import concourse.bass as bass
import concourse.bass_utils as _bass_utils
import concourse.mybir as mybir
import concourse.tile as tile
from concourse.bacc import Bacc
from concourse.bass_utils import run_bass_kernel_spmd

# Rewrite --enable-ldw-opt on the walrus_driver invocation.  Disabled: the
# walrus pass rejects the standalone InstLdweights that tile_legalize emits
# ("InstLdweights is not compatible with LDW optimization"); the redundant
# loads are excised directly in _excise_redundant_ldweights instead.
LDW_OPT = False


def _patch_walrus_ldw_opt():
    if getattr(_bass_utils, "_ldw_opt_patched", False):
        return
    orig = _bass_utils.run_command

    def run_command_ldw(argv, **kwargs):
        if (
            LDW_OPT
            and isinstance(argv, list)
            and any("walrus_driver" in str(a) for a in argv)
        ):
            argv = [
                "--enable-ldw-opt=true" if a == "--enable-ldw-opt=false" else a
                for a in argv
            ]
        return orig(argv, **kwargs)

    _bass_utils.run_command = run_command_ldw
    _bass_utils._ldw_opt_patched = True


_patch_walrus_ldw_opt()

N_CORES = 8
B, C, H, W = 64, 128, 56, 56
BPC = B // N_CORES  # images per core
KS, K = 3, 5
PH, PW = H + 2, W + 2  # zero-padded image
CHUNK_ROWS = 8
N_CHUNKS = H // CHUNK_ROWS
FREE = CHUNK_ROWS * W  # valid output elements per chunk (448)
FREE_R = CHUNK_ROWS * PW  # matmul free dim incl. garbage cols (464 <= 512)
F32 = mybir.dt.float32
F16 = mybir.dt.float16
BF16 = mybir.dt.bfloat16
F8 = mybir.dt.float8e4

# Elide the redundant LDWEIGHTS on matmuls 2..7 of each weight pass (the
# stationary tile is unchanged within a pass).
LDW_ELIDE = True


def _excise_redundant_ldweights(nc):
    """Remove InstLdweights whose stationary tile is already resident.

    tile_legalize pairs every non-f32 InstMatmult with a standalone
    InstLdweights; in the p-outer conv loop 6 of every 7 reload the identical
    weights.  Walrus emits no weight load for an InstMatmult with
    ldweights=False when its standalone load is gone, so the PE array keeps
    the resident weights and the matmul cadence drops from ~236ns
    (LDWEIGHTS-bound) to the raw fp8 DoubleRow rate.  Waits/updates of a
    removed load are merged into the following matmul; Bacc.compile()
    legalizes any wait overflow afterwards.
    """
    import concourse.mybir as _mb

    removed = 0
    for blk in nc.main_func.blocks:
        insts = list(blk.instructions)
        last_sig = None
        keep = []
        for idx, inst in enumerate(insts):
            if isinstance(inst, _mb.InstLdweights):
                a = inst.ins[0]
                sig = (
                    a.memref,
                    a.offset,
                    tuple(tuple(p) for p in a.ap),
                    str(a.dtype),
                )
                nxt = insts[idx + 1] if idx + 1 < len(insts) else None
                if (
                    sig == last_sig
                    and isinstance(nxt, _mb.InstMatmult)
                    and not nxt.is_transpose
                ):
                    si = inst.sync_info
                    if si is not None and (si.on_wait or si.on_update):
                        msi = nxt.sync_info
                        if msi is None:
                            nxt.sync_info = _mb.SyncInfo(
                                on_wait=list(si.on_wait),
                                on_update=list(si.on_update),
                            )
                        else:
                            nxt.sync_info = _mb.SyncInfo(
                                on_wait=list(msi.on_wait) + list(si.on_wait),
                                on_update=list(msi.on_update) + list(si.on_update),
                            )
                    try:
                        nxt.merge_dependencies_from(inst)
                    except Exception:
                        pass
                    removed += 1
                    continue  # drop this reload
                last_sig = sig
            elif isinstance(inst, _mb.InstMatmult):
                if inst.is_transpose:
                    last_sig = None
            keep.append(inst)
        if removed and len(keep) != len(insts):
            del blk.instructions[:]
            for inst in keep:
                blk.instructions.append(inst)
    return removed


def build_kernel(rv_vals):
    """Build the single-core Bass module (SPMD: same program on all 8 cores).

    rv_vals: the 5 rv scalars, baked as immediates into the weight-gen ops.
    """
    nc = Bacc()
    x_p = nc.declare_dram_parameter("x", [BPC, C, H, W], F16, isOutput=False)
    m_p = nc.declare_dram_parameter("M", [C, C, KS, KS], F32, isOutput=False)
    # Z rides as fp16: its contribution to w is scaled by rv (~4.5e-3), so
    # fp16 rounding perturbs w by ~3e-7 against w's sigma of 0.03 — ~0.3
    # expected weight-sign flips across all 147K weights (deterministic for
    # the fixed inputs; measured rel err stays ~2e-4).  M stays fp32 (it IS
    # w's magnitude; fp16 M would flip ~100x more signs).
    z_p = nc.declare_dram_parameter("Z", [K, C, C, KS, KS], F16, isOutput=False)
    a_p = nc.declare_dram_parameter("Alpha", [C, 1, 1], F32, isOutput=False)
    rv_p = nc.declare_dram_parameter("rv", [1, K], F32, isOutput=False)
    out_p = nc.declare_dram_parameter("out", [BPC, C, H, W], F16, isOutput=True)

    NW = C * KS * KS  # 1152 weight elements per out-channel row
    HALF = (H // 2) * W  # first-half image elements (28 rows)
    HGEN = NW // 2  # weight-gen half-column split (pipelines vs the z DMAs)

    with tile.TileContext(nc) as tc, ExitStack() as ctx:
        const = ctx.enter_context(tc.tile_pool(name="const", bufs=1))
        wg = ctx.enter_context(tc.tile_pool(name="wg", bufs=1))
        zpool = ctx.enter_context(tc.tile_pool(name="zpool", bufs=1))
        xin = ctx.enter_context(tc.tile_pool(name="xin", bufs=BPC))
        pad = ctx.enter_context(tc.tile_pool(name="pad", bufs=1))
        opool = ctx.enter_context(tc.tile_pool(name="opool", bufs=3))
        ps = ctx.enter_context(tc.tile_pool(name="ps", bufs=1, space="PSUM"))

        # ---- constants ----
        # Anti-diagonal permutation: transpose against it yields the transposed
        # tap with REVERSED out-channel columns, which is exactly the column
        # order DoubleRowSwInterleave's weight layout wants.
        # The tile name doubles as a NEFF-cache marker for the ldw-opt flag
        # (the cache keys on BIR content, not compiler flags).
        identity = const.tile([C, C], BF16, name=f"identity_ldw{int(LDW_OPT)}")
        nc.gpsimd.memset(identity[:], 0.0)
        nc.gpsimd.affine_select(
            out=identity[:],
            in_=identity[:],
            compare_op=mybir.AluOpType.not_equal,
            fill=1.0,
            base=-(C - 1),
            pattern=[[1, C]],
            channel_multiplier=1,
        )
        # Alpha/rv ride the ACT HWDGE ring so the SP ring's first slots
        # belong to x0/Z.
        alpha_sb = const.tile([C, 1], F32)
        nc.scalar.dma_start(alpha_sb[:], a_p[:].rearrange("c a b -> c (a b)"))
        rv_sb = const.tile([1, K], F32)
        nc.scalar.dma_start(rv_sb[:], rv_p[:])

        x_ap = x_p[:]
        o_ap = out_p[:]

        # The HWDGE SP ring drains FIFO, so this issue order is the wire
        # order.  The weight-gen STT chain paces to z_k arrivals (one STT per
        # landing), so Z goes out early, interleaved with image 0's halves
        # (whose signs must finish before the conv, ~16us in).  All remaining
        # images are issued up front (xin has BPC bufs, nothing recycles, the
        # ring never starves).
        # Wire order tuned so each landing's trailing compute hides inside
        # the remaining wire: z0-z3 (the DVE chain starts on z0 and is
        # throughput-bound after that), x0's halves (ACT signs run during the
        # z4/m wire), then z4, then M (folded into the chain LAST), then the
        # remaining images.  Whole-tensor z DMAs: half-DMAs doubled the
        # ~650ns-per-issue load on the SP sequencer for no wire gain.
        x_sbs = []
        x_sbs.append(xin.tile([C, H * W], F16, name="x_sb0", tag="x_sb"))
        z_sbs = []
        for k in range(K):
            z_sbs.append(zpool.tile([C, NW], F16, name=f"z{k}", tag=f"z{k}"))
        for k in range(K - 1):
            nc.sync.dma_start(
                z_sbs[k][:], z_p[k].rearrange("o i a b -> o (i a b)")
            )
        nc.sync.dma_start(
            x_sbs[0][:, 0:HALF], x_ap[0].rearrange("c h w -> c (h w)")[:, 0:HALF]
        )
        nc.sync.dma_start(
            x_sbs[0][:, HALF:], x_ap[0].rearrange("c h w -> c (h w)")[:, HALF:]
        )
        nc.sync.dma_start(
            z_sbs[K - 1][:], z_p[K - 1].rearrange("o i a b -> o (i a b)")
        )
        m_sb = wg.tile([C, NW], F32)
        nc.sync.dma_start(m_sb[:], m_p[:].rearrange("o i a b -> o (i a b)"))
        for i in range(1, BPC):
            x_sbs.append(xin.tile([C, H * W], F16, name=f"x_sb{i}", tag="x_sb"))
            nc.sync.dma_start(
                x_sbs[i][:], x_ap[i].rearrange("c h w -> c (h w)")
            )

        # ---- padded sign buffers: 3 physical buffers, borders zeroed ONCE.
        # Every image only writes the interior, so the zero border persists
        # across reuses.
        ba_bufs = []
        for b in range(3):
            ba = pad.tile([C, PH * PW + 2], F8, name=f"ba{b}", tag=f"ba{b}")
            ba_r = ba[:, 0 : PH * PW].rearrange("c (h w) -> c h w", w=PW)
            nc.gpsimd.memset(ba[:, 0:PW], 0.0)
            nc.gpsimd.memset(ba[:, (PH - 1) * PW : PH * PW + 2], 0.0)
            nc.gpsimd.memset(ba_r[:, 1 : H + 1, 0:1], 0.0)
            nc.gpsimd.memset(ba_r[:, 1 : H + 1, W + 1 : PW], 0.0)
            ba_bufs.append(ba)

        # ---- weight generation: w = (sum_k rv_k Z_k) + M  (DVE; Pool lacks
        # the TensorScalarPtr op).  Full-width ops (half-splitting only adds
        # op overhead — the chain is DVE-throughput-bound, ~8us); M is folded
        # LAST so its DMA can trail the (critical) Z wire.
        w_sb = wg.tile([C, NW], F32)
        nc.vector.tensor_scalar_mul(w_sb[:], z_sbs[0][:], float(rv_vals[0]))
        for k in range(1, K):
            nc.vector.scalar_tensor_tensor(
                w_sb[:],
                z_sbs[k][:],
                float(rv_vals[k]),
                w_sb[:],
                mybir.AluOpType.mult,
                mybir.AluOpType.add,
            )
        nc.vector.tensor_add(w_sb[:], w_sb[:], m_sb[:])
        bw_sb = wg.tile([C, NW], BF16)

        def psum_tile(ch, shape, dtype, name):
            # pt0 gets 2 banks so image i+1's first matmul needn't wait for
            # image i's chunk-0 eviction; 2 + 6 = 8 banks total.
            return ps.tile(
                shape, dtype, name=name, tag=f"pt{ch}", bufs=(2 if ch == 0 else 1)
            )

        def sign_image(i, halves=False):
            """Binarize image i's fp16 pixels into its ba buffer interior."""
            ba = ba_bufs[i % 3]
            ba_r = ba[:, 0 : PH * PW].rearrange("c (h w) -> c h w", w=PW)
            x_r = x_sbs[i][:].rearrange("c (h w) -> c h w", w=W)
            if halves:
                nc.scalar.sign(ba_r[:, 1 : H // 2 + 1, 1 : W + 1], x_r[:, : H // 2])
                nc.scalar.sign(ba_r[:, H // 2 + 1 : H + 1, 1 : W + 1], x_r[:, H // 2 :])
            else:
                nc.scalar.sign(ba_r[:, 1 : H + 1, 1 : W + 1], x_r)
            return ba

        # Image 0's sign runs as soon as its half-DMAs land, before the
        # weight sign (which waits on the Z chain) enters the ACT queue.
        sign_image(0, halves=True)
        nc.scalar.sign(bw_sb[:], w_sb[:])
        sign_image(1)

        # Transpose each tap's [oc, ic] into [ic, oc-reversed] (via the
        # anti-diagonal permutation), then interleave tap pairs column-wise as
        # fp8e4 (+-1 exact): the DoubleRowSwInterleave weight layout.  The
        # transposes borrow the conv's PSUM banks (idle during the prologue).
        # Chain every PE matmul (transposes included) in emission order with
        # ordering-only deps: the tile scheduler otherwise interleaves the
        # weight passes, breaking the same-weights runs the LDWEIGHTS
        # excision needs.
        pe_chain = [None]

        from concourse.instruction_name_ordered_set import (
            InstructionNameOrderedSet,
        )

        def chain_pe(bi):
            raw = bi.ins
            if pe_chain[0] is not None:
                s = InstructionNameOrderedSet()
                s.add(pe_chain[0])
                raw.add_nosync_dependencies_from(s)
            pe_chain[0] = raw.name

        wt = const.tile([C, 5, 2 * C], F8)
        nc.vector.memset(wt[:, 4, :], 0.0)
        bw_r = bw_sb[:].rearrange("o (i j) -> o i j", j=KS * KS)
        for j in range(KS * KS):
            tp = psum_tile(j % N_CHUNKS, [C, C], BF16, f"tp{j}")
            chain_pe(nc.tensor.transpose(tp[:], bw_r[:, :, j], identity[:]))
            pair, slot = divmod(j, 2)
            wt_h = wt[:].tensor
            dst = bass.AP(wt_h, pair * 2 * C + slot, [[5 * 2 * C, C], [2, C]])
            nc.vector.tensor_copy(dst, tp[:])
        # rv reaches the kernel as baked immediates; touch the tensor so the
        # bound input isn't dead.
        nc.vector.tensor_copy(w_sb[0:1, 0:K], rv_sb[0:1, :])

        def tap_off(r0, j):
            # flat offset of (out-row r0, tap j)'s top-left read in the padded image
            if j == KS * KS:  # zero tap: alias tap 8's window (weights are 0)
                j = KS * KS - 1
            return (r0 + j // KS) * PW + (j % KS)

        # Eviction engine per chunk: GpSimd has no PSUM port, so split
        # DVE/ScalarE; ScalarE also carries the signs.
        EVICT_DVE = (0, 2, 4, 6)

        def conv_image(i, ba):
            """5 weight passes x 7 chunk matmuls into 7 PSUM banks, then
            alpha-scaled eviction to fp16 and the output DMA."""
            pts = [
                psum_tile(ch, [C, 512], F32, f"pt{ch}_{i}") for ch in range(N_CHUNKS)
            ]
            for p in range(5):
                for ch in range(N_CHUNKS):
                    r0 = ch * CHUNK_ROWS
                    o0 = tap_off(r0, 2 * p)
                    o1 = tap_off(r0, 2 * p + 1)
                    rhs = bass.AP(
                        ba[:].tensor,
                        o0,
                        [[PH * PW + 2, C], [o1 - o0, 2], [1, FREE_R]],
                    )
                    mi = nc.tensor.matmul(
                        pts[ch][:, 0:FREE_R],
                        wt[:, p, :],
                        rhs,
                        start=(p == 0),
                        stop=(p == 4),
                        perf_mode=mybir.MatmulPerfMode.DoubleRowSwInterleave,
                    )
                    chain_pe(mi)
            o_sb = opool.tile([C, H * W], F16, name=f"o_sb{i}", tag="o_sb")
            for ch in range(N_CHUNKS):
                eng = nc.vector if ch in EVICT_DVE else nc.scalar
                src = pts[ch][:, 0:FREE_R].rearrange("c (a b) -> c a b", b=PW)[
                    :, :, 0:W
                ]
                dst = o_sb[:, ch * FREE : (ch + 1) * FREE].rearrange(
                    "c (a b) -> c a b", b=W
                )
                if ch in EVICT_DVE:
                    eng.tensor_scalar_mul(dst, src, alpha_sb[:, 0:1])
                else:
                    eng.mul(dst, src, alpha_sb[:, 0:1])
            return o_sb

        # Software-pipelined image loop.  Signs for images 0/1 were emitted
        # above; each iteration's sign(i+2) is emitted AFTER image i's
        # ScalarE evictions so it doesn't delay them in the ACT queue.
        for i in range(BPC):
            o_sb = conv_image(i, ba_bufs[i % 3])
            if i + 2 < BPC:
                sign_image(i + 2)
            o_hbm = o_ap[i].rearrange("c h w -> c (h w)")
            if i < 5:
                # Early outputs ride SWDGE (GpSimd) so they never head-of-line
                # block input loads on the FIFO HWDGE SP ring.
                nc.gpsimd.dma_start(o_hbm, o_sb[:])
            elif i < BPC - 1:
                # All input issues are done by ~20us; the SP ring is idle.
                nc.sync.dma_start(o_hbm, o_sb[:])
            else:
                # Final image drains in 2-chunk pieces, alternating the two
                # idle HWDGE rings, each issued as soon as its chunks evict.
                nc.sync.dma_start(o_hbm[:, 0 : 2 * FREE], o_sb[:, 0 : 2 * FREE])
                nc.scalar.dma_start(
                    o_hbm[:, 2 * FREE : 4 * FREE], o_sb[:, 2 * FREE : 4 * FREE]
                )
                nc.sync.dma_start(
                    o_hbm[:, 4 * FREE : 6 * FREE], o_sb[:, 4 * FREE : 6 * FREE]
                )
                nc.scalar.dma_start(o_hbm[:, 6 * FREE :], o_sb[:, 6 * FREE :])

    if LDW_ELIDE:
        _excise_redundant_ldweights(nc)
    nc.finalize()
    return nc


_CACHE = {}


def _get_nc(rv):
    key = rv.tobytes()
    if key not in _CACHE:
        _CACHE[key] = build_kernel(np.asarray(rv, np.float32).reshape(-1))
    return _CACHE[key]


def _run(inputs, trace=False):
    x = np.ascontiguousarray(np.asarray(inputs["x"]), dtype=np.float16)
    M = np.ascontiguousarray(np.asarray(inputs["M"], np.float32))
    Z = np.ascontiguousarray(np.asarray(inputs["Z"]), dtype=np.float16)
    Alpha = np.ascontiguousarray(np.asarray(inputs["Alpha"], np.float32))
    rv = np.ascontiguousarray(np.asarray(inputs["rv"], np.float32))
    nc = _get_nc(rv)
    in_maps = [
        {"x": x[c * BPC : (c + 1) * BPC], "M": M, "Z": Z, "Alpha": Alpha, "rv": rv}
        for c in range(N_CORES)
    ]
    res = run_bass_kernel_spmd(nc, in_maps, list(range(N_CORES)), trace=trace)
    out = np.concatenate([res.results[c]["out"] for c in range(N_CORES)], axis=0)
    return np.asarray(out, dtype=np.float32), res


def kernel(**inputs):
    out, _ = _run(inputs, trace=False)
    return out


def kernel_traced(**inputs):
    out, res = _run(inputs, trace=True)
    return out, res
